# revision 38
# baseline (speedup 1.0000x reference)
"""Multi-head attention forward (B=4, N=1024, D=768, H=12, dh=64) on 8 TRN2 cores.

Sharding: (batch, head-group) — core c handles batch b = c//2 and heads
hs..hs+5 where hs = (c%2)*6.  Each core computes its 6 heads' contribution
to out[b] = attn(x[b]) @ W_out_rows(for its heads); host sums the two
partials per batch and adds the bias (the "all-reduce after final linear").

All SBUF tensors are bf16 (halves DMA + keeps matmuls at 1 cycle/row for
any moving-dim size); PSUM accumulation stays fp32.

Per-core dataflow:
  qkT  [768,1024] = w_qk^T @ x^T          (d-major q,k — feeds scores;
                                           w_qk cols pair-packed
                                           [q_p0|k_p0|q_p1|k_p1|q_p2|k_p2])
  v    [1024,390] = x @ w_v (+ ones col)  (n-major v — feeds AV)
  S^T  [128k,2*512q]/(pair,i) = k_h @ q_h^T  (keys on partitions; the two
                                           heads' q-chunks side by side in
                                           one 2-bank PSUM tile)
  P^T  = exp(S^T * scale)                 (ONE [128,1024] ACT op per (pair,i):
                                           no max-sub — scores ~ N(0,1))
  o    [128q,65]/(head,qtile) = P_slice^T^T @ [v_h|1]  (q-major AV: full 128
                                           PE rows; col 64 = denominator)
  attn = o[:,0:64] * (1/o[:,64])          (per-partition scalar broadcast on
                                           DVE — no broadcast matmul needed)
  attT = PE-transpose(attn)               (back to d-major for the out proj)
  out  [1024,768] = attT^T @ w_o          (partial; host all-reduce)

PSUM accumulation groups zero a whole 2KB bank on start, so the 8 q-major
AV accumulators of a unit (4 per bank x 2 banks) must run sequentially
within each bank: the unit's AV is a 64-matmul burst executed in the NEXT
unit's window (PE is idle there waiting on the exp pipeline).  All deferred
PE work — AV bursts, transposes, projection / out-projection chain pieces —
flows through a FIFO fill queue popped between score matmuls, keeping the
tensor engine saturated at ~2-matmul granularity so the ACT exp cadence
never starves.
"""
import os
import sys

sys.path.insert(0, "/opt/trn_rl_repo")

# The kernel needs the axon-tunneled TRN2 PJRT backend; a JAX_PLATFORMS=cpu
# pin (common for reference-side jax) would hide the NeuronCores.
if os.environ.get("JAX_PLATFORMS", "").strip() == "cpu":
    del os.environ["JAX_PLATFORMS"]

import numpy as np
import concourse.bass as bass
import concourse.bacc as bacc
import concourse.tile as tile
from concourse import mybir
from concourse.bass_utils import run_bass_kernel_spmd
from contextlib import ExitStack

F32 = mybir.dt.float32
BF16 = mybir.dt.bfloat16

DIM = 768
N = 1024
HEADS_PER_CORE = 6
DH = 64
SCALE = DH ** -0.5
NCORES = 8
N_WARM = int(os.environ.get("ATTN_N_WARM", "32"))
STEP_BUDGET = float(os.environ.get("ATTN_STEP_BUDGET", "640"))


def build_nc():
    DT = BF16
    nc = bacc.Bacc("TRN2", target_bir_lowering=False, debug=False)

    xT_d = nc.declare_dram_parameter("xT", [DIM, N], DT, isOutput=False)
    wqk_d = nc.declare_dram_parameter("w_qk", [DIM, 768], DT, isOutput=False)
    wv_d = nc.declare_dram_parameter("w_v", [DIM, 384], DT, isOutput=False)
    wo_d = nc.declare_dram_parameter("w_o", [384, DIM], DT, isOutput=False)
    ones_d = nc.declare_dram_parameter("ones_col", [128, 64], DT, isOutput=False)
    ident_d = nc.declare_dram_parameter("ident", [128, 128], DT, isOutput=False)
    out_d = nc.declare_dram_parameter("out", [N, DIM], F32, isOutput=True)

    with tile.TileContext(nc) as tc:
        with ExitStack() as ctx:
            persist = ctx.enter_context(tc.tile_pool(name="persist", bufs=1))
            # pt ring sized for ~2.5 units in flight so a unit's AV burst may
            # spill into the unit after next without blocking the exp pipeline
            pt_pool = ctx.enter_context(tc.tile_pool(name="pt", bufs=20))
            attn_pool = ctx.enter_context(tc.tile_pool(name="attn", bufs=6))
            stats = ctx.enter_context(tc.tile_pool(name="stats", bufs=4))
            outsb = ctx.enter_context(tc.tile_pool(name="outsb", bufs=4))
            # PSUM slots are bank-granular: 2 x 2-bank score tiles + 2 x
            # 1-bank AV accumulators + 2 x 1-bank aux slots = all 8 banks.
            ps_s = ctx.enter_context(tc.tile_pool(name="ps_s", bufs=2, space="PSUM"))
            ps_acc = ctx.enter_context(tc.tile_pool(name="ps_acc", bufs=2, space="PSUM"))
            ps_aux = ctx.enter_context(tc.tile_pool(name="ps_aux", bufs=2, space="PSUM"))

            xT = persist.tile([128, 6, N], DT)
            wqk = persist.tile([128, 6, 768], DT)
            wv = persist.tile([128, 6, 384], DT)
            wo = persist.tile([128, 3, 768], DT)
            qkT = persist.tile([128, 6, N], DT)
            v_sb = persist.tile([128, 8, 6 * 65], DT)
            attT = persist.tile([128, 3, N], DT)
            ident = persist.tile([128, 128], DT)
            out_partial = persist.tile([128, 4, DIM], F32)
            warm_src = persist.tile([128, 256], DT)

            # Input DMAs, one batched transfer per tensor (each dma_start
            # costs ~0.6us on the shared HWDGE generator + ~0.9us sem
            # propagation, so fewer/bigger is strictly better).  Transfer
            # order on the exclusive DMA device: wqk pair-0 cols + xT first
            # (they gate the first projection chains), then w_v, the rest of
            # w_qk, w_o.  Two queues (SP + ACT) halve issue latency.
            # All input DMAs on the SP queue in strict priority order — the
            # DMA device is exclusive, so a second queue would only let a
            # low-priority transfer cut ahead of the chain-gating wqk/xT
            # pair (issue costs pipeline ahead of the transfers anyway).
            nc.sync.dma_start(
                out=wqk[:, :, 0:256],
                in_=wqk_d[:, 0:256].rearrange("(k p) n -> p k n", p=128),
            )
            nc.sync.dma_start(
                out=xT, in_=xT_d.rearrange("(k p) n -> p k n", p=128)
            )
            nc.sync.dma_start(
                out=wv, in_=wv_d.rearrange("(k p) n -> p k n", p=128)
            )
            nc.sync.dma_start(
                out=wqk[:, :, 256:768],
                in_=wqk_d[:, 256:768].rearrange("(k p) n -> p k n", p=128),
            )
            nc.sync.dma_start(
                out=wo, in_=wo_d.rearrange("(k p) n -> p k n", p=128)
            )
            ones_stage = persist.tile([128, 64], DT)
            nc.sync.dma_start(out=ones_stage, in_=ones_d[:, :])
            nc.sync.dma_start(out=ident, in_=ident_d[:, :])
            # ones: v_sb[:, i, h*65 + 64] = 1.0 for all (i, h). The strided
            # scatter runs on the otherwise-idle GPSIMD (SBUF->SBUF is its
            # one legal niche) so it can never head-of-line block DVE's
            # projection evictions; as a DMA it would cost 6144 min-size
            # descriptors.
            v_ones_view = v_sb.rearrange("p i (h c) -> p i h c", h=6)[:, :, :, 64]
            nc.gpsimd.tensor_copy(
                v_ones_view, ones_stage[:, 0:48].rearrange("p (i h) -> p i h", i=8)
            )

            # PE clock warm-up: the tensor engine idles during the input DMA
            # window and would otherwise start the projection at the cold
            # p-state (and the ramp needs 3us of continuous execution to
            # reach full clock).  Matmuls against a GPSIMD-memset SBUF tile
            # keep PE busy across the DMA window with no data dependencies;
            # the trailing N=1 ones keep the tail cheap so the first real
            # chain isn't delayed.
            nc.gpsimd.memset(warm_src, 1.0)
            # Preload the ACT exp table during the DMA window (first real exp
            # would otherwise pay the ~1.3us table load at weave start).
            warm_exp = stats.tile([128, 1], F32, tag="warm_exp", name="warm_exp")
            nc.scalar.activation(warm_exp, warm_src[:, 0:1],
                                 mybir.ActivationFunctionType.Exp, scale=SCALE)
            warm_ps = ps_aux.tile([1, 256], F32, tag="aux", name="warm_ps",
                                  padded_shape=[128, 512])
            for _w in range(N_WARM):
                nc.tensor.matmul(warm_ps, warm_src[:, 0:1], warm_src,
                                 start=True, stop=True)
            for _w in range(8):
                nc.tensor.matmul(warm_ps[:, 0:1], warm_src[:, 0:1],
                                 warm_src[:, 0:1], start=True, stop=True)

            # ---- deferred-PE-work fill queue ------------------------------
            # (cost_ns, closure) FIFO; attention units pop ~STEP_BUDGET ns of
            # work between score matmuls.  Emission order == pop order, so
            # enqueue order must respect data deps.
            fill_q = []

            def fq_push(cost, fn):
                fill_q.append((cost, fn))

            def fq_pop(budget):
                spent = 0.0
                while fill_q and spent < budget:
                    cost, fn = fill_q.pop(0)
                    fn()
                    spent += cost

            def fq_drain():
                while fill_q:
                    fill_q.pop(0)[1]()

            # ---- projection chains (as fine-grained queue items) ---------
            def chain(lhs_fn, rhs_fn, n_k, width, evict_fn, name):
                """Accumulating matmul chain split into 2-matmul queue items
                + an eviction item.  lhs_fn/rhs_fn map kt -> AP."""
                box = {}

                def piece(k0, k1):
                    def go():
                        if k0 == 0:
                            box["ps"] = ps_aux.tile(
                                [128, width], F32, tag="aux", name=name,
                                padded_shape=[128, 512])
                        ps = box["ps"]
                        for kt in range(k0, k1):
                            nc.tensor.matmul(
                                ps, lhs_fn(kt), rhs_fn(kt),
                                start=(kt == 0), stop=(kt == n_k - 1),
                            )
                    return go

                for k0 in range(0, n_k, 2):
                    k1 = min(k0 + 2, n_k)
                    fq_push((k1 - k0) * width * 0.42, piece(k0, k1))
                fq_push(0, lambda: evict_fn(box["ps"]))

            def qk_group(mt, chs=(0, 1), evict_on_act=False):
                """qkT[mt, ch-chunk] = (w_qk col-block mt)^T @ xT.
                Col blocks (pair-packed): mt=2p -> q of pair p, 2p+1 -> k."""
                for ch in chs:
                    def evict(ps, mt=mt, ch=ch):
                        dst = qkT[:, mt, ch * 512:(ch + 1) * 512]
                        if evict_on_act:
                            nc.scalar.copy(dst, ps)
                        else:
                            nc.vector.tensor_copy(dst, ps)
                    chain(
                        lambda kt, mt=mt: wqk[:, kt, mt * 128:(mt + 1) * 128],
                        lambda kt, ch=ch: xT[:, kt, ch * 512:(ch + 1) * 512],
                        6, 512, evict, f"qk_{mt}_{ch}",
                    )

            def v_group(i):
                """v rows-block i = x[i-block] @ w_v, strided into v_sb."""
                def evict(ps, i=i):
                    dst = v_sb[:, i, :].rearrange("p (h c) -> p h c", h=6)[:, :, 0:DH]
                    nc.vector.tensor_copy(dst, ps.rearrange("p (h c) -> p h c", h=6))
                chain(
                    lambda kt, i=i: xT[:, kt, i * 128:(i + 1) * 128],
                    lambda kt: wv[:, kt, :],
                    6, 384, evict, f"v_{i}",
                )

            # ---- attention ------------------------------------------------
            def emit_normalize_qt(p, ch, acc, qt, qi):
                """Single-qt normalize: groups (qi*2, qi*2+1) of `acc`."""
                dinv = stats.tile([128, 2], F32, tag="dinv",
                                  name=f"dinvq_{p}_{ch}_{qt}")
                dview = acc.rearrange("p (g c) -> p g c", g=4)[:, 2 * qi:2 * qi + 2, 64]
                with nc.allow_low_precision(
                    reason="softmax denominators are O(100); rounding of "
                    "1/denom is below the bf16 noise floor of the weave"
                ):
                    nc.vector.reciprocal(dinv, dview)
                stage = attn_pool.tile(
                    [128, 128], BF16, tag="attn", name=f"attnq_{p}_{ch}_{qt}"
                )
                for hp in range(2):
                    nc.vector.tensor_scalar_mul(
                        stage[:, hp * 64:hp * 64 + 64],
                        acc[:, (2 * qi + hp) * 128:(2 * qi + hp) * 128 + 64],
                        dinv[:, hp:hp + 1],
                    )
                return stage

            def emit_normalize(p, ch, acc, qts):
                """acc holds 4 groups (qt, hp) at 128-col stride, col 64 of
                each group = softmax denominator.  DVE: one strided
                reciprocal + 4 per-partition-scalar muls into bf16 staging."""
                dinv = stats.tile([128, 4], F32, tag="dinv",
                                  name=f"dinv_{p}_{ch}_{qts[0]}")
                dview = acc.rearrange("p (g c) -> p g c", g=4)[:, :, 64]
                with nc.allow_low_precision(
                    reason="softmax denominators are O(100); rounding of "
                    "1/denom is below the bf16 noise floor of the weave"
                ):
                    nc.vector.reciprocal(dinv, dview)
                stages = []
                for qi, qt in enumerate(qts):
                    stage = attn_pool.tile(
                        [128, 128], BF16, tag="attn", name=f"attn_{p}_{ch}_{qt}"
                    )
                    for hp in range(2):
                        g = qi * 2 + hp
                        nc.vector.tensor_scalar_mul(
                            stage[:, hp * 64:hp * 64 + 64],
                            acc[:, g * 128:g * 128 + 64],
                            dinv[:, g:g + 1],
                        )
                    stages.append((qt, stage))
                return stages

            def emit_transpose(p, ch, qt, stage, evict_act=False):
                """PE-transpose one [128q, 128d(2 heads)] staging tile into
                d-major attT columns; bf16 PSUM via bitcast.  Eviction on DVE
                (weave) or ACT (tail, where ACT is idle and DVE is the
                serial bottleneck)."""
                tp_f32 = ps_aux.tile([128, 64], F32, tag="aux",
                                     name=f"tp_{p}_{ch}_{qt}",
                                     padded_shape=[128, 512])
                tp = tp_f32.bitcast(BF16)
                nc.tensor.matmul(tp, stage, ident, is_transpose=True,
                                 start=True, stop=True)
                dst = attT[:, p, ch * 512 + qt * 128:ch * 512 + (qt + 1) * 128]
                if evict_act:
                    nc.scalar.copy(dst, tp)
                else:
                    nc.vector.tensor_copy(dst, tp)

            def enqueue_av(prev, post_x=(), post_y=()):
                """Enqueue the AV burst + normalize + transposes for a
                finished unit.  PSUM groups zero a whole bank on start, so
                the 4 (qt,hp) groups of each bank run sequentially (each
                fully accumulated over i=0..7 before the next starts).
                `post_x`/`post_y` callbacks push follow-up work right behind
                each bank's normalize+transposes (used by the tail so each
                output row's final fires as soon as its attT columns land)."""
                pp, pch, ppts = prev
                boxes = {}

                def alloc(bank):
                    def go():
                        boxes[bank] = ps_acc.tile(
                            [128, 512], F32, tag="acc", name=f"acc{bank}_{pp}_{pch}"
                        )
                    return go

                def av_group(bank, g, qt, hp):
                    def go():
                        acc = boxes[bank]
                        h = 2 * pp + hp
                        for i in range(8):
                            nc.tensor.matmul(
                                acc[:, g * 128:g * 128 + 65],
                                ppts[i][:, hp * 512 + qt * 128:hp * 512 + (qt + 1) * 128],
                                v_sb[:, i, h * 65:h * 65 + 65],
                                start=(i == 0),
                                stop=(i == 7),
                            )
                    return go

                stage_box = {}

                def norm(bank, qts):
                    def go():
                        stage_box[bank] = emit_normalize(pp, pch, boxes[bank], qts)
                    return go

                def tp(bank):
                    def go():
                        for qt, stage in stage_box[bank]:
                            emit_transpose(pp, pch, qt, stage)
                    return go

                # The normalize item carries an inflated cost so the pop
                # loop breaks between it and the transposes — the next score
                # matmul then fills the PE pipeline while the DVE muls run
                # (the transposes read their output, so emitting them
                # back-to-back would head-of-line block PE on DVE latency).
                fq_push(0, alloc("X"))
                for g, (qt, hp) in enumerate(((0, 0), (0, 1), (1, 0), (1, 1))):
                    fq_push(8 * 65 * 0.42, av_group("X", g, qt, hp))
                fq_push(STEP_BUDGET, norm("X", (0, 1)))
                fq_push(STEP_BUDGET, lambda: None)  # 2nd score between norm/tp
                fq_push(110, tp("X"))
                for fn in post_x:
                    fn()
                fq_push(0, alloc("Y"))
                for g, (qt, hp) in enumerate(((2, 0), (2, 1), (3, 0), (3, 1))):
                    fq_push(8 * 65 * 0.42, av_group("Y", g, qt, hp))
                fq_push(STEP_BUDGET, norm("Y", (2, 3)))
                fq_push(STEP_BUDGET, lambda: None)
                fq_push(110, tp("Y"))
                for fn in post_y:
                    fn()

            def enqueue_av_tail(prev, finals):
                """Tail variant of enqueue_av: per-qt stagger so each output
                row's final (j=2 + add + DMA) fires as soon as that qt's two
                AV groups, normalize, and transpose land — instead of
                waiting for a whole bank of four."""
                pp, pch, ppts = prev
                boxes = {}

                def alloc(bank):
                    def go():
                        boxes[bank] = ps_acc.tile(
                            [128, 512], F32, tag="acc", name=f"acct{bank}_{pp}_{pch}"
                        )
                    return go

                def av_group(bank, g, qt, hp):
                    def go():
                        acc = boxes[bank]
                        h = 2 * pp + hp
                        for i in range(8):
                            nc.tensor.matmul(
                                acc[:, g * 128:g * 128 + 65],
                                ppts[i][:, hp * 512 + qt * 128:hp * 512 + (qt + 1) * 128],
                                v_sb[:, i, h * 65:h * 65 + 65],
                                start=(i == 0),
                                stop=(i == 7),
                            )
                    return go

                stage_box = {}

                def norm_qt(bank, qt, qi):
                    def go():
                        stage_box[qt] = emit_normalize_qt(pp, pch, boxes[bank], qt, qi)
                    return go

                def tp_qt(qt):
                    def go():
                        emit_transpose(pp, pch, qt, stage_box[qt], evict_act=True)
                    return go

                # Software-pipelined: qt+1's AV groups sit between qt's
                # normalize (DVE) and its transpose (PE reads the DVE
                # output), so PE never head-of-line blocks on DVE latency.
                # Bank X holds qt0/qt1's groups, bank Y qt2/qt3's; a bank's
                # second qt starts only after its first stopped (sequential
                # accumulation groups), and the normalize reads survive the
                # later start because PSUM zeroing is lazy.
                plan = []
                for qt in range(4):
                    bank = "X" if qt < 2 else "Y"
                    qi = qt % 2
                    items = []
                    if qi == 0:
                        items.append((0, alloc(bank)))
                    for hp in range(2):
                        items.append((8 * 65 * 0.42, av_group(bank, 2 * qi + hp, qt, hp)))
                    items.append((0, norm_qt(bank, qt, qi)))
                    plan.append(items)
                    plan.append([(60, tp_qt(qt)), (0, finals[qt])])
                # interleave: AV(qt+1) between norm(qt) and tp(qt)
                order = [0, 2, 1, 4, 3, 6, 5, 7]
                for idx in order:
                    for cost, fn in plan[idx]:
                        fq_push(cost, fn)

            def attention_unit(p, ch, prev, extra=()):
                """One (pair, query-chunk) unit: 8 x (scores both heads into
                a 2-bank PSUM tile, one 1024-wide exp).  The previous unit's
                AV/normalize/transposes are enqueued first so they fill this
                unit's PE bubbles (the scores pipeline is ACT-bound).
                `extra` closures are enqueued after the AV items — for work
                that depends on the previous unit's attT transposes."""
                qt_ = 2 * p       # qkT tile of this pair's q
                kt_ = 2 * p + 1   # qkT tile of this pair's k
                pts = []

                def score_exp(i):
                    s = ps_s.tile([128, 1024], F32, tag="s", name=f"s_{p}_{ch}_{i}")
                    for hp in range(2):
                        lo, hi = hp * 64, hp * 64 + 64
                        nc.tensor.matmul(
                            s[:, hp * 512:(hp + 1) * 512],
                            qkT[lo:hi, kt_, i * 128:(i + 1) * 128],
                            qkT[lo:hi, qt_, ch * 512:(ch + 1) * 512],
                            start=True,
                            stop=True,
                        )
                    pt = pt_pool.tile([128, 1024], BF16, tag="pt",
                                      name=f"pt_{p}_{ch}_{i}")
                    nc.scalar.activation(pt, s, mybir.ActivationFunctionType.Exp,
                                         scale=SCALE)
                    pts.append(pt)

                score_exp(0)
                score_exp(1)
                if prev is not None:
                    enqueue_av(prev)
                for fn in extra:
                    fn()
                for i in range(2, 8):
                    score_exp(i)
                    fq_pop(STEP_BUDGET)
                fq_pop(2 * STEP_BUDGET)
                return (p, ch, pts)

            # ---- out-projection ------------------------------------------
            CHUNKS = ((0, 512), (512, 256))
            _osb_cache = {}

            def out_group(i, c):
                """Chunk c of row-block i (rows 0..3 path): 3 j-matmuls +
                DVE evict; c==1 flushes the row's 768-wide DMA."""
                c0, cw = CHUNKS[c]
                if c == 0:
                    osb = outsb.tile([128, 768], F32, tag="osb", name=f"osb_{i}")
                    _osb_cache[i] = osb
                else:
                    osb = _osb_cache.pop(i)

                def evict(ps, i=i, c=c, osb=osb):
                    nc.vector.tensor_copy(osb[:, c0:c0 + cw], ps)
                    if c == 1:
                        nc.sync.dma_start(
                            out=out_d[i * 128:(i + 1) * 128, :], in_=osb)
                chain(
                    lambda j, i=i: attT[:, j, i * 128:(i + 1) * 128],
                    lambda j, c0=c0, cw=cw: wo[:, j, c0:c0 + cw],
                    3, cw, evict, f"o_{i}_{c}",
                )

            def out_partial_chain(i, c):
                """j=0,1 of (row-block i, chunk c) into out_partial (rows
                4..7 path; ACT evict — ACT is idle post-weave)."""
                c0, cw = CHUNKS[c]

                def evict(ps, i=i):
                    nc.scalar.copy(out_partial[:, i - 4, c0:c0 + cw], ps)
                chain(
                    lambda j, i=i: attT[:, j, i * 128:(i + 1) * 128],
                    lambda j, c0=c0, cw=cw: wo[:, j, c0:c0 + cw],
                    2, cw, evict, f"pp_{i}_{c}",
                )

            def out_final(i):
                """j=2 matmuls + adds of the precomputed j=0,1 partials for
                row-block i, flushed with a single 768-wide DMA.  Emits
                directly (tail context: the fill queue is being drained, and
                inline emission keeps each row's DMA as early as possible)."""
                osb = outsb.tile([128, 768], F32, tag="osb", name=f"osbf_{i}")
                for c, (c0, cw) in enumerate(CHUNKS):
                    ps = ps_aux.tile([128, cw], F32, tag="aux", name=f"f_{i}_{c}",
                                     padded_shape=[128, 512])
                    nc.tensor.matmul(
                        ps,
                        attT[:, 2, i * 128:(i + 1) * 128],
                        wo[:, 2, c0:c0 + cw],
                        start=True,
                        stop=True,
                    )
                    nc.vector.tensor_add(
                        osb[:, c0:c0 + cw], ps,
                        out_partial[:, i - 4, c0:c0 + cw],
                    )
                    if i == 7:
                        # flush per chunk so the kernel-end drain only waits
                        # on the short final transfer
                        nc.sync.dma_start(
                            out=out_d[i * 128:(i + 1) * 128, c0:c0 + cw],
                            in_=osb[:, c0:c0 + cw],
                        )
                if i != 7:
                    nc.sync.dma_start(out=out_d[i * 128:(i + 1) * 128, :], in_=osb)

            # ---- schedule -------------------------------------------------
            # Pre-weave: the minimum projections unit (0,0) needs — q pair 0
            # chunk 0 and k pair 0 both chunks (scores sweep all key blocks).
            # Alternating eviction engines so neither DVE nor ACT gates the
            # first scores.  q chunk 1 flows through the queue.
            qk_group(0, chs=(0,))
            qk_group(1, evict_on_act=True)
            fq_drain()

            # Weave. Query-chunk-0 units first: once (0,0),(1,0),(2,0) are
            # done, output row-blocks 0..3 are fully determined, so their
            # out-projection (and DMA) overlaps the chunk-1 units.  v, the
            # remaining qk projections, AV bursts, and out chains all flow
            # through the fill queue; a full drain before each unit bounds
            # the backlog and guarantees every unit's inputs (its pair's qkT
            # chains, v for the AV bursts) are emitted before its scores.
            qk_group(0, chs=(1,))
            for i in range(8):
                v_group(i)
            prev = attention_unit(0, 0, None)
            qk_group(2)
            qk_group(3)
            fq_drain()
            prev = attention_unit(1, 0, prev)
            qk_group(4)
            qk_group(5)
            fq_drain()
            prev = attention_unit(2, 0, prev)
            fq_drain()
            # out row-blocks 0..3 depend only on the ch-0 attT columns; the
            # last of those (pair 2) is transposed inside unit (0,1) by its
            # AV items, so the row chains can fill units (0,1) and (1,1) —
            # without them the late units' fill queues run dry and PE both
            # idles and drops to the cold p-state.
            prev = attention_unit(0, 1, prev, extra=[
                lambda i=i, c=c: out_group(i, c)
                for i in (0, 1) for c in (0, 1)
            ])
            fq_drain()
            prev = attention_unit(1, 1, prev, extra=[
                lambda i=i, c=c: out_group(i, c)
                for i in (2, 3) for c in (0, 1)
            ])
            fq_drain()
            # rows 4..7 j=0,1 partials read attT pair-0/1 ch-1; pair 1 is
            # only transposed inside unit (2,1) — enqueue them behind its AV
            # items.  The j=2 finals then ride the tail AV's post hooks so
            # each row's add+DMA fires as soon as its attT columns land.
            prev = attention_unit(2, 1, prev, extra=[
                lambda i=i, c=c: out_partial_chain(i, c)
                for i in (4, 5, 6, 7) for c in (0, 1)
            ])
            enqueue_av_tail(prev, [lambda i=i: out_final(i) for i in (4, 5, 6, 7)])
            fq_drain()

    nc.compile()
    return nc


_NC_CACHE = {}


def _get_nc():
    if "nc" not in _NC_CACHE:
        _NC_CACHE["nc"] = build_nc()
    return _NC_CACHE["nc"]


def kernel(x, w_qkv, w_out, b_out):
    import ml_dtypes

    def bf16(a):
        return np.ascontiguousarray(
            np.asarray(a, dtype=np.float32).astype(ml_dtypes.bfloat16)
        )

    x = np.asarray(x, dtype=np.float32)
    w_qkv = np.asarray(w_qkv, dtype=np.float32)
    w_out = np.asarray(w_out, dtype=np.float32)
    b_out = np.asarray(b_out, dtype=np.float32)

    nc = _get_nc()
    ones_col = np.ones((128, 64), dtype=np.float32)
    ident = np.eye(128, dtype=np.float32)
    in_maps = []
    for c in range(NCORES):
        b = c // 2
        hs = (c % 2) * HEADS_PER_CORE
        q_cols = w_qkv[:, hs * DH:(hs + 6) * DH]
        k_cols = w_qkv[:, 768 + hs * DH:768 + (hs + 6) * DH]
        # pair-packed: [q_p0 | k_p0 | q_p1 | k_p1 | q_p2 | k_p2], 128 each
        wqk_packed = np.concatenate(
            [blk for p in range(3)
             for blk in (q_cols[:, p * 128:(p + 1) * 128],
                         k_cols[:, p * 128:(p + 1) * 128])],
            axis=1,
        )
        in_maps.append({
            "xT": bf16(x[b].T),
            "w_qk": bf16(wqk_packed),
            "w_v": bf16(w_qkv[:, 1536 + hs * DH:1536 + (hs + 6) * DH]),
            "w_o": bf16(w_out[hs * DH:(hs + 6) * DH, :]),
            "ones_col": bf16(ones_col),
            "ident": bf16(ident),
        })

    res = run_bass_kernel_spmd(nc, in_maps, core_ids=list(range(NCORES))).results

    out = np.empty((4, N, DIM), dtype=np.float32)
    for b in range(4):
        out[b] = res[2 * b]["out"] + res[2 * b + 1]["out"] + b_out
    return out


# revision 41
# speedup vs baseline: 1.0030x; 1.0030x over previous
"""Multi-head attention forward (B=4, N=1024, D=768, H=12, dh=64) on 8 TRN2 cores.

Sharding: (batch, head-group) — core c handles batch b = c//2 and heads
hs..hs+5 where hs = (c%2)*6.  Each core computes its 6 heads' contribution
to out[b] = attn(x[b]) @ W_out_rows(for its heads); host sums the two
partials per batch and adds the bias (the "all-reduce after final linear").

All SBUF tensors are bf16 (halves DMA + keeps matmuls at 1 cycle/row for
any moving-dim size); PSUM accumulation stays fp32.

Per-core dataflow:
  qkT  [768,1024] = w_qk^T @ x^T          (d-major q,k — feeds scores;
                                           w_qk cols pair-packed
                                           [q_p0|k_p0|q_p1|k_p1|q_p2|k_p2])
  v    [1024,390] = x @ w_v (+ ones col)  (n-major v — feeds AV)
  S^T  [128k,2*512q]/(pair,i) = k_h @ q_h^T  (keys on partitions; the two
                                           heads' q-chunks side by side in
                                           one 2-bank PSUM tile)
  P^T  = exp(S^T * scale)                 (ONE [128,1024] ACT op per (pair,i):
                                           no max-sub — scores ~ N(0,1))
  o    [128q,65]/(head,qtile) = P_slice^T^T @ [v_h|1]  (q-major AV: full 128
                                           PE rows; col 64 = denominator)
  attn = o[:,0:64] * (1/o[:,64])          (per-partition scalar broadcast on
                                           DVE — no broadcast matmul needed)
  attT = PE-transpose(attn)               (back to d-major for the out proj)
  out  [1024,768] = attT^T @ w_o          (partial; host all-reduce)

PSUM accumulation groups zero a whole 2KB bank on start, so the 8 q-major
AV accumulators of a unit (4 per bank x 2 banks) must run sequentially
within each bank: the unit's AV is a 64-matmul burst executed in the NEXT
unit's window (PE is idle there waiting on the exp pipeline).  All deferred
PE work — AV bursts, transposes, projection / out-projection chain pieces —
flows through a FIFO fill queue popped between score matmuls, keeping the
tensor engine saturated at ~2-matmul granularity so the ACT exp cadence
never starves.
"""
import os
import sys

sys.path.insert(0, "/opt/trn_rl_repo")

# The kernel needs the axon-tunneled TRN2 PJRT backend; a JAX_PLATFORMS=cpu
# pin (common for reference-side jax) would hide the NeuronCores.
if os.environ.get("JAX_PLATFORMS", "").strip() == "cpu":
    del os.environ["JAX_PLATFORMS"]

import numpy as np
import concourse.bass as bass
import concourse.bacc as bacc
import concourse.tile as tile
from concourse import mybir
from concourse.bass_utils import run_bass_kernel_spmd
from contextlib import ExitStack

F32 = mybir.dt.float32
BF16 = mybir.dt.bfloat16

DIM = 768
N = 1024
HEADS_PER_CORE = 6
DH = 64
SCALE = DH ** -0.5
NCORES = 8
N_WARM = int(os.environ.get("ATTN_N_WARM", "32"))
STEP_BUDGET = float(os.environ.get("ATTN_STEP_BUDGET", "640"))


def build_nc():
    DT = BF16
    nc = bacc.Bacc("TRN2", target_bir_lowering=False, debug=False)

    xT_d = nc.declare_dram_parameter("xT", [DIM, N], DT, isOutput=False)
    wqk_d = nc.declare_dram_parameter("w_qk", [DIM, 768], DT, isOutput=False)
    wv_d = nc.declare_dram_parameter("w_v", [DIM, 384], DT, isOutput=False)
    wo_d = nc.declare_dram_parameter("w_o", [384, DIM], DT, isOutput=False)
    ones_d = nc.declare_dram_parameter("ones_col", [128, 64], DT, isOutput=False)
    ident_d = nc.declare_dram_parameter("ident", [128, 128], DT, isOutput=False)
    out_d = nc.declare_dram_parameter("out", [N, DIM], F32, isOutput=True)

    with tile.TileContext(nc) as tc:
        with ExitStack() as ctx:
            persist = ctx.enter_context(tc.tile_pool(name="persist", bufs=1))
            # pt ring sized for ~2.5 units in flight so a unit's AV burst may
            # spill into the unit after next without blocking the exp pipeline
            pt_pool = ctx.enter_context(tc.tile_pool(name="pt", bufs=20))
            attn_pool = ctx.enter_context(tc.tile_pool(name="attn", bufs=6))
            stats = ctx.enter_context(tc.tile_pool(name="stats", bufs=4))
            outsb = ctx.enter_context(tc.tile_pool(name="outsb", bufs=4))
            # PSUM slots are bank-granular: 2 x 2-bank score tiles + 2 x
            # 1-bank AV accumulators + 2 x 1-bank aux slots = all 8 banks.
            ps_s = ctx.enter_context(tc.tile_pool(name="ps_s", bufs=2, space="PSUM"))
            ps_acc = ctx.enter_context(tc.tile_pool(name="ps_acc", bufs=2, space="PSUM"))
            ps_aux = ctx.enter_context(tc.tile_pool(name="ps_aux", bufs=2, space="PSUM"))

            xT = persist.tile([128, 6, N], DT)
            wqk = persist.tile([128, 6, 768], DT)
            wv = persist.tile([128, 6, 384], DT)
            wo = persist.tile([128, 3, 768], DT)
            qkT = persist.tile([128, 6, N], DT)
            v_sb = persist.tile([128, 8, 6 * 65], DT)
            attT = persist.tile([128, 3, N], DT)
            ident = persist.tile([128, 128], DT)
            out_partial = persist.tile([128, 4, DIM], F32)
            warm_src = persist.tile([128, 256], DT)

            # Input DMAs, one batched transfer per tensor (each dma_start
            # costs ~0.6us on the shared HWDGE generator + ~0.9us sem
            # propagation, so fewer/bigger is strictly better).  Transfer
            # order on the exclusive DMA device: wqk pair-0 cols + xT first
            # (they gate the first projection chains), then w_v, the rest of
            # w_qk, w_o.  Two queues (SP + ACT) halve issue latency.
            # All input DMAs on the SP queue in strict priority order — the
            # DMA device is exclusive, so a second queue would only let a
            # low-priority transfer cut ahead of the chain-gating wqk/xT
            # pair (issue costs pipeline ahead of the transfers anyway).
            nc.sync.dma_start(
                out=wqk[:, :, 0:256],
                in_=wqk_d[:, 0:256].rearrange("(k p) n -> p k n", p=128),
            )
            nc.sync.dma_start(
                out=xT, in_=xT_d.rearrange("(k p) n -> p k n", p=128)
            )
            nc.sync.dma_start(
                out=wv, in_=wv_d.rearrange("(k p) n -> p k n", p=128)
            )
            nc.sync.dma_start(
                out=wqk[:, :, 256:768],
                in_=wqk_d[:, 256:768].rearrange("(k p) n -> p k n", p=128),
            )
            nc.sync.dma_start(
                out=wo, in_=wo_d.rearrange("(k p) n -> p k n", p=128)
            )
            ones_stage = persist.tile([128, 64], DT)
            nc.sync.dma_start(out=ones_stage, in_=ones_d[:, :])
            nc.sync.dma_start(out=ident, in_=ident_d[:, :])
            # ones: v_sb[:, i, h*65 + 64] = 1.0 for all (i, h). The strided
            # scatter runs on the otherwise-idle GPSIMD (SBUF->SBUF is its
            # one legal niche) so it can never head-of-line block DVE's
            # projection evictions; as a DMA it would cost 6144 min-size
            # descriptors.
            v_ones_view = v_sb.rearrange("p i (h c) -> p i h c", h=6)[:, :, :, 64]
            nc.gpsimd.tensor_copy(
                v_ones_view, ones_stage[:, 0:48].rearrange("p (i h) -> p i h", i=8)
            )

            # PE clock warm-up: the tensor engine idles during the input DMA
            # window and would otherwise start the projection at the cold
            # p-state (and the ramp needs 3us of continuous execution to
            # reach full clock).  Matmuls against a GPSIMD-memset SBUF tile
            # keep PE busy across the DMA window with no data dependencies;
            # the trailing N=1 ones keep the tail cheap so the first real
            # chain isn't delayed.
            nc.gpsimd.memset(warm_src, 1.0)
            # Preload the ACT exp table during the DMA window (first real exp
            # would otherwise pay the ~1.3us table load at weave start).
            warm_exp = stats.tile([128, 1], F32, tag="warm_exp", name="warm_exp")
            nc.scalar.activation(warm_exp, warm_src[:, 0:1],
                                 mybir.ActivationFunctionType.Exp, scale=SCALE)
            warm_ps = ps_aux.tile([1, 256], F32, tag="aux", name="warm_ps",
                                  padded_shape=[128, 512])
            for _w in range(N_WARM):
                nc.tensor.matmul(warm_ps, warm_src[:, 0:1], warm_src,
                                 start=True, stop=True)
            for _w in range(8):
                nc.tensor.matmul(warm_ps[:, 0:1], warm_src[:, 0:1],
                                 warm_src[:, 0:1], start=True, stop=True)

            # ---- deferred-PE-work fill queue ------------------------------
            # (cost_ns, closure) FIFO; attention units pop ~STEP_BUDGET ns of
            # work between score matmuls.  Emission order == pop order, so
            # enqueue order must respect data deps.
            fill_q = []

            def fq_push(cost, fn):
                fill_q.append((cost, fn))

            def fq_pop(budget):
                spent = 0.0
                while fill_q and spent < budget:
                    cost, fn = fill_q.pop(0)
                    fn()
                    spent += cost

            def fq_drain():
                while fill_q:
                    fill_q.pop(0)[1]()

            # ---- projection chains (as fine-grained queue items) ---------
            def chain(lhs_fn, rhs_fn, n_k, width, evict_fn, name):
                """Accumulating matmul chain split into 2-matmul queue items
                + an eviction item.  lhs_fn/rhs_fn map kt -> AP."""
                box = {}

                def piece(k0, k1):
                    def go():
                        if k0 == 0:
                            box["ps"] = ps_aux.tile(
                                [128, width], F32, tag="aux", name=name,
                                padded_shape=[128, 512])
                        ps = box["ps"]
                        for kt in range(k0, k1):
                            nc.tensor.matmul(
                                ps, lhs_fn(kt), rhs_fn(kt),
                                start=(kt == 0), stop=(kt == n_k - 1),
                            )
                    return go

                for k0 in range(0, n_k, 2):
                    k1 = min(k0 + 2, n_k)
                    fq_push((k1 - k0) * width * 0.42, piece(k0, k1))
                fq_push(0, lambda: evict_fn(box["ps"]))

            def qk_group(mt, chs=(0, 1), evict_on_act=False):
                """qkT[mt, ch-chunk] = (w_qk col-block mt)^T @ xT.
                Col blocks (pair-packed): mt=2p -> q of pair p, 2p+1 -> k."""
                for ch in chs:
                    def evict(ps, mt=mt, ch=ch):
                        dst = qkT[:, mt, ch * 512:(ch + 1) * 512]
                        if evict_on_act:
                            nc.scalar.copy(dst, ps)
                        else:
                            nc.vector.tensor_copy(dst, ps)
                    chain(
                        lambda kt, mt=mt: wqk[:, kt, mt * 128:(mt + 1) * 128],
                        lambda kt, ch=ch: xT[:, kt, ch * 512:(ch + 1) * 512],
                        6, 512, evict, f"qk_{mt}_{ch}",
                    )

            def v_group(i):
                """v rows-block i = x[i-block] @ w_v, strided into v_sb."""
                def evict(ps, i=i):
                    dst = v_sb[:, i, :].rearrange("p (h c) -> p h c", h=6)[:, :, 0:DH]
                    nc.vector.tensor_copy(dst, ps.rearrange("p (h c) -> p h c", h=6))
                chain(
                    lambda kt, i=i: xT[:, kt, i * 128:(i + 1) * 128],
                    lambda kt: wv[:, kt, :],
                    6, 384, evict, f"v_{i}",
                )

            # ---- attention ------------------------------------------------
            def emit_normalize_qt(p, ch, acc, qt, qi):
                """Single-qt normalize: groups (qi*2, qi*2+1) of `acc`."""
                dinv = stats.tile([128, 2], F32, tag="dinv",
                                  name=f"dinvq_{p}_{ch}_{qt}")
                dview = acc.rearrange("p (g c) -> p g c", g=4)[:, 2 * qi:2 * qi + 2, 64]
                with nc.allow_low_precision(
                    reason="softmax denominators are O(100); rounding of "
                    "1/denom is below the bf16 noise floor of the weave"
                ):
                    nc.vector.reciprocal(dinv, dview)
                stage = attn_pool.tile(
                    [128, 128], BF16, tag="attn", name=f"attnq_{p}_{ch}_{qt}"
                )
                for hp in range(2):
                    nc.vector.tensor_scalar_mul(
                        stage[:, hp * 64:hp * 64 + 64],
                        acc[:, (2 * qi + hp) * 128:(2 * qi + hp) * 128 + 64],
                        dinv[:, hp:hp + 1],
                    )
                return stage

            def emit_normalize(p, ch, acc, qts):
                """acc holds 4 groups (qt, hp) at 128-col stride, col 64 of
                each group = softmax denominator.  DVE: one strided
                reciprocal + 4 per-partition-scalar muls into bf16 staging."""
                dinv = stats.tile([128, 4], F32, tag="dinv",
                                  name=f"dinv_{p}_{ch}_{qts[0]}")
                dview = acc.rearrange("p (g c) -> p g c", g=4)[:, :, 64]
                with nc.allow_low_precision(
                    reason="softmax denominators are O(100); rounding of "
                    "1/denom is below the bf16 noise floor of the weave"
                ):
                    nc.vector.reciprocal(dinv, dview)
                stages = []
                for qi, qt in enumerate(qts):
                    stage = attn_pool.tile(
                        [128, 128], BF16, tag="attn", name=f"attn_{p}_{ch}_{qt}"
                    )
                    for hp in range(2):
                        g = qi * 2 + hp
                        nc.vector.tensor_scalar_mul(
                            stage[:, hp * 64:hp * 64 + 64],
                            acc[:, g * 128:g * 128 + 64],
                            dinv[:, g:g + 1],
                        )
                    stages.append((qt, stage))
                return stages

            def emit_transpose(p, ch, qt, stage, evict_act=False):
                """PE-transpose one [128q, 128d(2 heads)] staging tile into
                d-major attT columns; bf16 PSUM via bitcast.  Eviction on DVE
                (weave) or ACT (tail, where ACT is idle and DVE is the
                serial bottleneck)."""
                tp_f32 = ps_aux.tile([128, 64], F32, tag="aux",
                                     name=f"tp_{p}_{ch}_{qt}",
                                     padded_shape=[128, 512])
                tp = tp_f32.bitcast(BF16)
                nc.tensor.matmul(tp, stage, ident, is_transpose=True,
                                 start=True, stop=True)
                dst = attT[:, p, ch * 512 + qt * 128:ch * 512 + (qt + 1) * 128]
                if evict_act:
                    nc.scalar.copy(dst, tp)
                else:
                    nc.vector.tensor_copy(dst, tp)

            def enqueue_av(prev, post_x=(), post_y=()):
                """Enqueue the AV burst + normalize + transposes for a
                finished unit.  PSUM groups zero a whole bank on start, so
                the 4 (qt,hp) groups of each bank run sequentially (each
                fully accumulated over i=0..7 before the next starts).
                `post_x`/`post_y` callbacks push follow-up work right behind
                each bank's normalize+transposes (used by the tail so each
                output row's final fires as soon as its attT columns land)."""
                pp, pch, ppts = prev
                boxes = {}

                def alloc(bank):
                    def go():
                        boxes[bank] = ps_acc.tile(
                            [128, 512], F32, tag="acc", name=f"acc{bank}_{pp}_{pch}"
                        )
                    return go

                def av_group(bank, g, qt, hp):
                    def go():
                        acc = boxes[bank]
                        h = 2 * pp + hp
                        for i in range(8):
                            nc.tensor.matmul(
                                acc[:, g * 128:g * 128 + 65],
                                ppts[i][:, hp * 512 + qt * 128:hp * 512 + (qt + 1) * 128],
                                v_sb[:, i, h * 65:h * 65 + 65],
                                start=(i == 0),
                                stop=(i == 7),
                            )
                    return go

                stage_box = {}

                def norm(bank, qts):
                    def go():
                        stage_box[bank] = emit_normalize(pp, pch, boxes[bank], qts)
                    return go

                def tp(bank):
                    def go():
                        for qt, stage in stage_box[bank]:
                            emit_transpose(pp, pch, qt, stage)
                    return go

                # The normalize item carries an inflated cost so the pop
                # loop breaks between it and the transposes — the next score
                # matmul then fills the PE pipeline while the DVE muls run
                # (the transposes read their output, so emitting them
                # back-to-back would head-of-line block PE on DVE latency).
                fq_push(0, alloc("X"))
                for g, (qt, hp) in enumerate(((0, 0), (0, 1), (1, 0), (1, 1))):
                    fq_push(8 * 65 * 0.42, av_group("X", g, qt, hp))
                fq_push(STEP_BUDGET, norm("X", (0, 1)))
                fq_push(110, tp("X"))
                for fn in post_x:
                    fn()
                fq_push(0, alloc("Y"))
                for g, (qt, hp) in enumerate(((2, 0), (2, 1), (3, 0), (3, 1))):
                    fq_push(8 * 65 * 0.42, av_group("Y", g, qt, hp))
                fq_push(STEP_BUDGET, norm("Y", (2, 3)))
                fq_push(110, tp("Y"))
                for fn in post_y:
                    fn()

            def enqueue_av_tail(prev, finals):
                """Tail variant of enqueue_av: per-qt stagger so each output
                row's final (j=2 + add + DMA) fires as soon as that qt's two
                AV groups, normalize, and transpose land — instead of
                waiting for a whole bank of four."""
                pp, pch, ppts = prev
                boxes = {}

                def alloc(bank):
                    def go():
                        boxes[bank] = ps_acc.tile(
                            [128, 512], F32, tag="acc", name=f"acct{bank}_{pp}_{pch}"
                        )
                    return go

                def av_group(bank, g, qt, hp):
                    def go():
                        acc = boxes[bank]
                        h = 2 * pp + hp
                        for i in range(8):
                            nc.tensor.matmul(
                                acc[:, g * 128:g * 128 + 65],
                                ppts[i][:, hp * 512 + qt * 128:hp * 512 + (qt + 1) * 128],
                                v_sb[:, i, h * 65:h * 65 + 65],
                                start=(i == 0),
                                stop=(i == 7),
                            )
                    return go

                stage_box = {}

                def norm_qt(bank, qt, qi):
                    def go():
                        stage_box[qt] = emit_normalize_qt(pp, pch, boxes[bank], qt, qi)
                    return go

                def tp_qt(qt):
                    def go():
                        emit_transpose(pp, pch, qt, stage_box[qt], evict_act=True)
                    return go

                # Software-pipelined: qt+1's AV groups sit between qt's
                # normalize (DVE) and its transpose (PE reads the DVE
                # output), so PE never head-of-line blocks on DVE latency.
                # Bank X holds qt0/qt1's groups, bank Y qt2/qt3's; a bank's
                # second qt starts only after its first stopped (sequential
                # accumulation groups), and the normalize reads survive the
                # later start because PSUM zeroing is lazy.
                plan = []
                for qt in range(4):
                    bank = "X" if qt < 2 else "Y"
                    qi = qt % 2
                    items = []
                    if qi == 0:
                        items.append((0, alloc(bank)))
                    for hp in range(2):
                        items.append((8 * 65 * 0.42, av_group(bank, 2 * qi + hp, qt, hp)))
                    items.append((0, norm_qt(bank, qt, qi)))
                    plan.append(items)
                    plan.append([(60, tp_qt(qt)), (0, finals[qt])])
                # interleave: AV(qt+1) between norm(qt) and tp(qt)
                order = [0, 2, 1, 4, 3, 6, 5, 7]
                for idx in order:
                    for cost, fn in plan[idx]:
                        fq_push(cost, fn)

            def attention_unit(p, ch, prev, extra=()):
                """One (pair, query-chunk) unit: 8 x (scores both heads into
                a 2-bank PSUM tile, one 1024-wide exp).  The previous unit's
                AV/normalize/transposes are enqueued first so they fill this
                unit's PE bubbles (the scores pipeline is ACT-bound).
                `extra` closures are enqueued after the AV items — for work
                that depends on the previous unit's attT transposes."""
                qt_ = 2 * p       # qkT tile of this pair's q
                kt_ = 2 * p + 1   # qkT tile of this pair's k
                pts = []

                def score_exp(i):
                    s = ps_s.tile([128, 1024], F32, tag="s", name=f"s_{p}_{ch}_{i}")
                    for hp in range(2):
                        lo, hi = hp * 64, hp * 64 + 64
                        nc.tensor.matmul(
                            s[:, hp * 512:(hp + 1) * 512],
                            qkT[lo:hi, kt_, i * 128:(i + 1) * 128],
                            qkT[lo:hi, qt_, ch * 512:(ch + 1) * 512],
                            start=True,
                            stop=True,
                        )
                    pt = pt_pool.tile([128, 1024], BF16, tag="pt",
                                      name=f"pt_{p}_{ch}_{i}")
                    nc.scalar.activation(pt, s, mybir.ActivationFunctionType.Exp,
                                         scale=SCALE)
                    pts.append(pt)

                score_exp(0)
                score_exp(1)
                if prev is not None:
                    enqueue_av(prev)
                for fn in extra:
                    fn()
                for i in range(2, 8):
                    score_exp(i)
                    fq_pop(STEP_BUDGET)
                fq_pop(2 * STEP_BUDGET)
                return (p, ch, pts)

            # ---- out-projection ------------------------------------------
            CHUNKS = ((0, 512), (512, 256))
            _osb_cache = {}

            def out_group(i, c):
                """Chunk c of row-block i (rows 0..3 path): 3 j-matmuls +
                DVE evict; c==1 flushes the row's 768-wide DMA."""
                c0, cw = CHUNKS[c]
                if c == 0:
                    osb = outsb.tile([128, 768], F32, tag="osb", name=f"osb_{i}")
                    _osb_cache[i] = osb
                else:
                    osb = _osb_cache.pop(i)

                def evict(ps, i=i, c=c, osb=osb):
                    nc.vector.tensor_copy(osb[:, c0:c0 + cw], ps)
                    if c == 1:
                        nc.sync.dma_start(
                            out=out_d[i * 128:(i + 1) * 128, :], in_=osb)
                chain(
                    lambda j, i=i: attT[:, j, i * 128:(i + 1) * 128],
                    lambda j, c0=c0, cw=cw: wo[:, j, c0:c0 + cw],
                    3, cw, evict, f"o_{i}_{c}",
                )

            def out_partial_chain(i, c):
                """j=0,1 of (row-block i, chunk c) into out_partial (rows
                4..7 path; ACT evict — ACT is idle post-weave)."""
                c0, cw = CHUNKS[c]

                def evict(ps, i=i):
                    nc.scalar.copy(out_partial[:, i - 4, c0:c0 + cw], ps)
                chain(
                    lambda j, i=i: attT[:, j, i * 128:(i + 1) * 128],
                    lambda j, c0=c0, cw=cw: wo[:, j, c0:c0 + cw],
                    2, cw, evict, f"pp_{i}_{c}",
                )

            def out_final(i):
                """j=2 matmuls + adds of the precomputed j=0,1 partials for
                row-block i, flushed with a single 768-wide DMA.  Emits
                directly (tail context: the fill queue is being drained, and
                inline emission keeps each row's DMA as early as possible)."""
                osb = outsb.tile([128, 768], F32, tag="osb", name=f"osbf_{i}")
                for c, (c0, cw) in enumerate(CHUNKS):
                    ps = ps_aux.tile([128, cw], F32, tag="aux", name=f"f_{i}_{c}",
                                     padded_shape=[128, 512])
                    nc.tensor.matmul(
                        ps,
                        attT[:, 2, i * 128:(i + 1) * 128],
                        wo[:, 2, c0:c0 + cw],
                        start=True,
                        stop=True,
                    )
                    nc.vector.tensor_add(
                        osb[:, c0:c0 + cw], ps,
                        out_partial[:, i - 4, c0:c0 + cw],
                    )
                    if i == 7:
                        # flush per chunk so the kernel-end drain only waits
                        # on the short final transfer
                        nc.sync.dma_start(
                            out=out_d[i * 128:(i + 1) * 128, c0:c0 + cw],
                            in_=osb[:, c0:c0 + cw],
                        )
                if i != 7:
                    nc.sync.dma_start(out=out_d[i * 128:(i + 1) * 128, :], in_=osb)

            # ---- schedule -------------------------------------------------
            # Pre-weave: the minimum projections unit (0,0) needs — q pair 0
            # chunk 0 and k pair 0 both chunks (scores sweep all key blocks).
            # Alternating eviction engines so neither DVE nor ACT gates the
            # first scores.  q chunk 1 flows through the queue.
            qk_group(0, chs=(0,))
            qk_group(1, evict_on_act=True)
            fq_drain()

            # Weave. Query-chunk-0 units first: once (0,0),(1,0),(2,0) are
            # done, output row-blocks 0..3 are fully determined, so their
            # out-projection (and DMA) overlaps the chunk-1 units.  v, the
            # remaining qk projections, AV bursts, and out chains all flow
            # through the fill queue; a full drain before each unit bounds
            # the backlog and guarantees every unit's inputs (its pair's qkT
            # chains, v for the AV bursts) are emitted before its scores.
            qk_group(0, chs=(1,))
            for i in range(8):
                v_group(i)
            prev = attention_unit(0, 0, None)
            qk_group(2)
            qk_group(3)
            fq_drain()
            prev = attention_unit(1, 0, prev)
            qk_group(4)
            qk_group(5)
            fq_drain()
            prev = attention_unit(2, 0, prev)
            fq_drain()
            # out row-blocks 0..3 depend only on the ch-0 attT columns; the
            # last of those (pair 2) is transposed inside unit (0,1) by its
            # AV items, so the row chains can fill units (0,1) and (1,1) —
            # without them the late units' fill queues run dry and PE both
            # idles and drops to the cold p-state.
            prev = attention_unit(0, 1, prev, extra=[
                lambda i=i, c=c: out_group(i, c)
                for i in (0, 1) for c in (0, 1)
            ])
            fq_drain()
            prev = attention_unit(1, 1, prev, extra=[
                lambda i=i, c=c: out_group(i, c)
                for i in (2, 3) for c in (0, 1)
            ])
            fq_drain()
            # rows 4..7 j=0,1 partials read attT pair-0/1 ch-1; pair 1 is
            # only transposed inside unit (2,1) — enqueue them behind its AV
            # items.  The j=2 finals then ride the tail AV's post hooks so
            # each row's add+DMA fires as soon as its attT columns land.
            prev = attention_unit(2, 1, prev, extra=[
                lambda i=i, c=c: out_partial_chain(i, c)
                for i in (4, 5) for c in (0, 1)
            ])
            for i in (6, 7):
                for c in (0, 1):
                    out_partial_chain(i, c)
            enqueue_av_tail(prev, [lambda i=i: out_final(i) for i in (4, 5, 6, 7)])
            fq_drain()

    nc.compile()
    return nc


_NC_CACHE = {}


def _get_nc():
    if "nc" not in _NC_CACHE:
        _NC_CACHE["nc"] = build_nc()
    return _NC_CACHE["nc"]


def kernel(x, w_qkv, w_out, b_out):
    import ml_dtypes

    def bf16(a):
        return np.ascontiguousarray(
            np.asarray(a, dtype=np.float32).astype(ml_dtypes.bfloat16)
        )

    x = np.asarray(x, dtype=np.float32)
    w_qkv = np.asarray(w_qkv, dtype=np.float32)
    w_out = np.asarray(w_out, dtype=np.float32)
    b_out = np.asarray(b_out, dtype=np.float32)

    nc = _get_nc()
    ones_col = np.ones((128, 64), dtype=np.float32)
    ident = np.eye(128, dtype=np.float32)
    in_maps = []
    for c in range(NCORES):
        b = c // 2
        hs = (c % 2) * HEADS_PER_CORE
        q_cols = w_qkv[:, hs * DH:(hs + 6) * DH]
        k_cols = w_qkv[:, 768 + hs * DH:768 + (hs + 6) * DH]
        # pair-packed: [q_p0 | k_p0 | q_p1 | k_p1 | q_p2 | k_p2], 128 each
        wqk_packed = np.concatenate(
            [blk for p in range(3)
             for blk in (q_cols[:, p * 128:(p + 1) * 128],
                         k_cols[:, p * 128:(p + 1) * 128])],
            axis=1,
        )
        in_maps.append({
            "xT": bf16(x[b].T),
            "w_qk": bf16(wqk_packed),
            "w_v": bf16(w_qkv[:, 1536 + hs * DH:1536 + (hs + 6) * DH]),
            "w_o": bf16(w_out[hs * DH:(hs + 6) * DH, :]),
            "ones_col": bf16(ones_col),
            "ident": bf16(ident),
        })

    res = run_bass_kernel_spmd(nc, in_maps, core_ids=list(range(NCORES))).results

    out = np.empty((4, N, DIM), dtype=np.float32)
    for b in range(4):
        out[b] = res[2 * b]["out"] + res[2 * b + 1]["out"] + b_out
    return out


# revision 45
# speedup vs baseline: 1.0214x; 1.0184x over previous
"""Multi-head attention forward (B=4, N=1024, D=768, H=12, dh=64) on 8 TRN2 cores.

Sharding: (batch, head-group) — core c handles batch b = c//2 and heads
hs..hs+5 where hs = (c%2)*6.  Each core computes its 6 heads' contribution
to out[b] = attn(x[b]) @ W_out_rows(for its heads); host sums the two
partials per batch and adds the bias (the "all-reduce after final linear").

All SBUF tensors are bf16 (halves DMA + keeps matmuls at 1 cycle/row for
any moving-dim size); PSUM accumulation stays fp32.

Per-core dataflow:
  qkT  [768,1024] = w_qk^T @ x^T          (d-major q,k — feeds scores;
                                           w_qk cols pair-packed
                                           [q_p0|k_p0|q_p1|k_p1|q_p2|k_p2])
  v    [1024,390] = x @ w_v (+ ones col)  (n-major v — feeds AV)
  S^T  [128k,2*512q]/(pair,i) = k_h @ q_h^T  (keys on partitions; the two
                                           heads' q-chunks side by side in
                                           one 2-bank PSUM tile)
  P^T  = exp(S^T * scale)                 (ONE [128,1024] ACT op per (pair,i):
                                           no max-sub — scores ~ N(0,1))
  o    [128q,65]/(head,qtile) = P_slice^T^T @ [v_h|1]  (q-major AV: full 128
                                           PE rows; col 64 = denominator)
  attn = o[:,0:64] * (1/o[:,64])          (per-partition scalar broadcast on
                                           DVE — no broadcast matmul needed)
  attT = PE-transpose(attn)               (back to d-major for the out proj)
  out  [1024,768] = attT^T @ w_o          (partial; host all-reduce)

PSUM accumulation groups zero a whole 2KB bank on start, so the 8 q-major
AV accumulators of a unit (4 per bank x 2 banks) must run sequentially
within each bank: the unit's AV is a 64-matmul burst executed in the NEXT
unit's window (PE is idle there waiting on the exp pipeline).  All deferred
PE work — AV bursts, transposes, projection / out-projection chain pieces —
flows through a FIFO fill queue popped between score matmuls, keeping the
tensor engine saturated at ~2-matmul granularity so the ACT exp cadence
never starves.
"""
import os
import sys

sys.path.insert(0, "/opt/trn_rl_repo")

# The kernel needs the axon-tunneled TRN2 PJRT backend; a JAX_PLATFORMS=cpu
# pin (common for reference-side jax) would hide the NeuronCores.
if os.environ.get("JAX_PLATFORMS", "").strip() == "cpu":
    del os.environ["JAX_PLATFORMS"]

import numpy as np
import concourse.bass as bass
import concourse.bacc as bacc
import concourse.tile as tile
from concourse import mybir
from concourse.bass_utils import run_bass_kernel_spmd
from contextlib import ExitStack

F32 = mybir.dt.float32
BF16 = mybir.dt.bfloat16

DIM = 768
N = 1024
HEADS_PER_CORE = 6
DH = 64
SCALE = DH ** -0.5
NCORES = 8
N_WARM = int(os.environ.get("ATTN_N_WARM", "32"))
STEP_BUDGET = float(os.environ.get("ATTN_STEP_BUDGET", "640"))


def build_nc():
    DT = BF16
    nc = bacc.Bacc("TRN2", target_bir_lowering=False, debug=False)

    xT_d = nc.declare_dram_parameter("xT", [DIM, N], DT, isOutput=False)
    wqk_d = nc.declare_dram_parameter("w_qk", [DIM, 768], DT, isOutput=False)
    wv_d = nc.declare_dram_parameter("w_v", [DIM, 384], DT, isOutput=False)
    wo_d = nc.declare_dram_parameter("w_o", [384, DIM], DT, isOutput=False)
    ones_d = nc.declare_dram_parameter("ones_col", [128, 64], DT, isOutput=False)
    ident_d = nc.declare_dram_parameter("ident", [128, 128], DT, isOutput=False)
    out_d = nc.declare_dram_parameter("out", [N, DIM], F32, isOutput=True)
    # rows 4..7 leave the device in two pieces summed on the host: j=0,1
    # partials (DMA'd during the last unit) go to out, the j=2 remainder
    # (tail) to out2 — killing the on-device combining adds, which were the
    # serial-DVE bottleneck of the tail.
    out2_d = nc.declare_dram_parameter("out2", [N // 2, DIM], F32, isOutput=True)

    with tile.TileContext(nc) as tc:
        with ExitStack() as ctx:
            persist = ctx.enter_context(tc.tile_pool(name="persist", bufs=1))
            # pt ring sized for ~2.5 units in flight so a unit's AV burst may
            # spill into the unit after next without blocking the exp pipeline
            pt_pool = ctx.enter_context(tc.tile_pool(name="pt", bufs=20))
            attn_pool = ctx.enter_context(tc.tile_pool(name="attn", bufs=6))
            stats = ctx.enter_context(tc.tile_pool(name="stats", bufs=4))
            outsb = ctx.enter_context(tc.tile_pool(name="outsb", bufs=4))
            # PSUM slots are bank-granular: 2 x 2-bank score tiles + 2 x
            # 1-bank AV accumulators + 2 x 1-bank aux slots = all 8 banks.
            ps_s = ctx.enter_context(tc.tile_pool(name="ps_s", bufs=2, space="PSUM"))
            ps_acc = ctx.enter_context(tc.tile_pool(name="ps_acc", bufs=2, space="PSUM"))
            ps_aux = ctx.enter_context(tc.tile_pool(name="ps_aux", bufs=2, space="PSUM"))

            xT = persist.tile([128, 6, N], DT)
            wqk = persist.tile([128, 6, 768], DT)
            wv = persist.tile([128, 6, 384], DT)
            wo = persist.tile([128, 3, 768], DT)
            qkT = persist.tile([128, 6, N], DT)
            v_sb = persist.tile([128, 8, 6 * 65], DT)
            attT = persist.tile([128, 3, N], DT)
            ident = persist.tile([128, 128], DT)
            out_partial = persist.tile([128, 4, DIM], F32)
            warm_src = persist.tile([128, 256], DT)

            # Input DMAs, one batched transfer per tensor (each dma_start
            # costs ~0.6us on the shared HWDGE generator + ~0.9us sem
            # propagation, so fewer/bigger is strictly better).  Transfer
            # order on the exclusive DMA device: wqk pair-0 cols + xT first
            # (they gate the first projection chains), then w_v, the rest of
            # w_qk, w_o.  Two queues (SP + ACT) halve issue latency.
            # All input DMAs on the SP queue in strict priority order — the
            # DMA device is exclusive, so a second queue would only let a
            # low-priority transfer cut ahead of the chain-gating wqk/xT
            # pair (issue costs pipeline ahead of the transfers anyway).
            nc.sync.dma_start(
                out=wqk[:, :, 0:256],
                in_=wqk_d[:, 0:256].rearrange("(k p) n -> p k n", p=128),
            )
            nc.sync.dma_start(
                out=xT, in_=xT_d.rearrange("(k p) n -> p k n", p=128)
            )
            nc.sync.dma_start(
                out=wv, in_=wv_d.rearrange("(k p) n -> p k n", p=128)
            )
            nc.sync.dma_start(
                out=wqk[:, :, 256:768],
                in_=wqk_d[:, 256:768].rearrange("(k p) n -> p k n", p=128),
            )
            nc.sync.dma_start(
                out=wo, in_=wo_d.rearrange("(k p) n -> p k n", p=128)
            )
            ones_stage = persist.tile([128, 64], DT)
            nc.sync.dma_start(out=ones_stage, in_=ones_d[:, :])
            nc.sync.dma_start(out=ident, in_=ident_d[:, :])
            # ones: v_sb[:, i, h*65 + 64] = 1.0 for all (i, h). The strided
            # scatter runs on the otherwise-idle GPSIMD (SBUF->SBUF is its
            # one legal niche) so it can never head-of-line block DVE's
            # projection evictions; as a DMA it would cost 6144 min-size
            # descriptors.
            v_ones_view = v_sb.rearrange("p i (h c) -> p i h c", h=6)[:, :, :, 64]
            nc.gpsimd.tensor_copy(
                v_ones_view, ones_stage[:, 0:48].rearrange("p (i h) -> p i h", i=8)
            )

            # PE clock warm-up: the tensor engine idles during the input DMA
            # window and would otherwise start the projection at the cold
            # p-state (and the ramp needs 3us of continuous execution to
            # reach full clock).  Matmuls against a GPSIMD-memset SBUF tile
            # keep PE busy across the DMA window with no data dependencies;
            # the trailing N=1 ones keep the tail cheap so the first real
            # chain isn't delayed.
            nc.gpsimd.memset(warm_src, 1.0)
            # Preload the ACT exp table during the DMA window (first real exp
            # would otherwise pay the ~1.3us table load at weave start).
            warm_exp = stats.tile([128, 1], F32, tag="warm_exp", name="warm_exp")
            nc.scalar.activation(warm_exp, warm_src[:, 0:1],
                                 mybir.ActivationFunctionType.Exp, scale=SCALE)
            warm_ps = ps_aux.tile([1, 256], F32, tag="aux", name="warm_ps",
                                  padded_shape=[128, 512])
            for _w in range(N_WARM):
                nc.tensor.matmul(warm_ps, warm_src[:, 0:1], warm_src,
                                 start=True, stop=True)
            for _w in range(8):
                nc.tensor.matmul(warm_ps[:, 0:1], warm_src[:, 0:1],
                                 warm_src[:, 0:1], start=True, stop=True)

            # ---- deferred-PE-work fill queue ------------------------------
            # (cost_ns, closure) FIFO; attention units pop ~STEP_BUDGET ns of
            # work between score matmuls.  Emission order == pop order, so
            # enqueue order must respect data deps.
            fill_q = []

            def fq_push(cost, fn):
                fill_q.append((cost, fn))

            def fq_pop(budget):
                spent = 0.0
                while fill_q and spent < budget:
                    cost, fn = fill_q.pop(0)
                    fn()
                    spent += cost

            def fq_drain():
                while fill_q:
                    fill_q.pop(0)[1]()

            # ---- projection chains (as fine-grained queue items) ---------
            def chain(lhs_fn, rhs_fn, n_k, width, evict_fn, name):
                """Accumulating matmul chain split into 2-matmul queue items
                + an eviction item.  lhs_fn/rhs_fn map kt -> AP."""
                box = {}

                def piece(k0, k1):
                    def go():
                        if k0 == 0:
                            box["ps"] = ps_aux.tile(
                                [128, width], F32, tag="aux", name=name,
                                padded_shape=[128, 512])
                        ps = box["ps"]
                        for kt in range(k0, k1):
                            nc.tensor.matmul(
                                ps, lhs_fn(kt), rhs_fn(kt),
                                start=(kt == 0), stop=(kt == n_k - 1),
                            )
                    return go

                for k0 in range(0, n_k, 2):
                    k1 = min(k0 + 2, n_k)
                    fq_push((k1 - k0) * width * 0.42, piece(k0, k1))
                fq_push(0, lambda: evict_fn(box["ps"]))

            def qk_group(mt, chs=(0, 1), evict_on_act=False):
                """qkT[mt, ch-chunk] = (w_qk col-block mt)^T @ xT.
                Col blocks (pair-packed): mt=2p -> q of pair p, 2p+1 -> k."""
                for ch in chs:
                    def evict(ps, mt=mt, ch=ch):
                        dst = qkT[:, mt, ch * 512:(ch + 1) * 512]
                        if evict_on_act:
                            nc.scalar.copy(dst, ps)
                        else:
                            nc.vector.tensor_copy(dst, ps)
                    chain(
                        lambda kt, mt=mt: wqk[:, kt, mt * 128:(mt + 1) * 128],
                        lambda kt, ch=ch: xT[:, kt, ch * 512:(ch + 1) * 512],
                        6, 512, evict, f"qk_{mt}_{ch}",
                    )

            def v_group(i):
                """v rows-block i = x[i-block] @ w_v, strided into v_sb."""
                def evict(ps, i=i):
                    dst = v_sb[:, i, :].rearrange("p (h c) -> p h c", h=6)[:, :, 0:DH]
                    nc.vector.tensor_copy(dst, ps.rearrange("p (h c) -> p h c", h=6))
                chain(
                    lambda kt, i=i: xT[:, kt, i * 128:(i + 1) * 128],
                    lambda kt: wv[:, kt, :],
                    6, 384, evict, f"v_{i}",
                )

            # ---- attention ------------------------------------------------
            def emit_normalize_qt(p, ch, acc, qt, qi):
                """Single-qt normalize: groups (qi*2, qi*2+1) of `acc`."""
                dinv = stats.tile([128, 2], F32, tag="dinv",
                                  name=f"dinvq_{p}_{ch}_{qt}")
                dview = acc.rearrange("p (g c) -> p g c", g=4)[:, 2 * qi:2 * qi + 2, 64]
                with nc.allow_low_precision(
                    reason="softmax denominators are O(100); rounding of "
                    "1/denom is below the bf16 noise floor of the weave"
                ):
                    nc.vector.reciprocal(dinv, dview)
                stage = attn_pool.tile(
                    [128, 128], BF16, tag="attn", name=f"attnq_{p}_{ch}_{qt}"
                )
                for hp in range(2):
                    nc.vector.tensor_scalar_mul(
                        stage[:, hp * 64:hp * 64 + 64],
                        acc[:, (2 * qi + hp) * 128:(2 * qi + hp) * 128 + 64],
                        dinv[:, hp:hp + 1],
                    )
                return stage

            def emit_normalize(p, ch, acc, qts):
                """acc holds 4 groups (qt, hp) at 128-col stride, col 64 of
                each group = softmax denominator.  DVE: one strided
                reciprocal + 4 per-partition-scalar muls into bf16 staging."""
                dinv = stats.tile([128, 4], F32, tag="dinv",
                                  name=f"dinv_{p}_{ch}_{qts[0]}")
                dview = acc.rearrange("p (g c) -> p g c", g=4)[:, :, 64]
                with nc.allow_low_precision(
                    reason="softmax denominators are O(100); rounding of "
                    "1/denom is below the bf16 noise floor of the weave"
                ):
                    nc.vector.reciprocal(dinv, dview)
                stages = []
                for qi, qt in enumerate(qts):
                    stage = attn_pool.tile(
                        [128, 128], BF16, tag="attn", name=f"attn_{p}_{ch}_{qt}"
                    )
                    for hp in range(2):
                        g = qi * 2 + hp
                        nc.vector.tensor_scalar_mul(
                            stage[:, hp * 64:hp * 64 + 64],
                            acc[:, g * 128:g * 128 + 64],
                            dinv[:, g:g + 1],
                        )
                    stages.append((qt, stage))
                return stages

            def emit_transpose(p, ch, qt, stage, evict_act=False):
                """PE-transpose one [128q, 128d(2 heads)] staging tile into
                d-major attT columns; bf16 PSUM via bitcast.  Eviction on DVE
                (weave) or ACT (tail, where ACT is idle and DVE is the
                serial bottleneck)."""
                tp_f32 = ps_aux.tile([128, 64], F32, tag="aux",
                                     name=f"tp_{p}_{ch}_{qt}",
                                     padded_shape=[128, 512])
                tp = tp_f32.bitcast(BF16)
                nc.tensor.matmul(tp, stage, ident, is_transpose=True,
                                 start=True, stop=True)
                dst = attT[:, p, ch * 512 + qt * 128:ch * 512 + (qt + 1) * 128]
                if evict_act:
                    nc.scalar.copy(dst, tp)
                else:
                    nc.vector.tensor_copy(dst, tp)

            def enqueue_av(prev, post_x=(), post_y=()):
                """Enqueue the AV burst + normalize + transposes for a
                finished unit.  PSUM groups zero a whole bank on start, so
                the 4 (qt,hp) groups of each bank run sequentially (each
                fully accumulated over i=0..7 before the next starts).
                `post_x`/`post_y` callbacks push follow-up work right behind
                each bank's normalize+transposes (used by the tail so each
                output row's final fires as soon as its attT columns land)."""
                pp, pch, ppts = prev
                boxes = {}

                def alloc(bank):
                    def go():
                        boxes[bank] = ps_acc.tile(
                            [128, 512], F32, tag="acc", name=f"acc{bank}_{pp}_{pch}"
                        )
                    return go

                def av_group(bank, g, qt, hp):
                    def go():
                        acc = boxes[bank]
                        h = 2 * pp + hp
                        for i in range(8):
                            nc.tensor.matmul(
                                acc[:, g * 128:g * 128 + 65],
                                ppts[i][:, hp * 512 + qt * 128:hp * 512 + (qt + 1) * 128],
                                v_sb[:, i, h * 65:h * 65 + 65],
                                start=(i == 0),
                                stop=(i == 7),
                            )
                    return go

                stage_box = {}

                def norm(bank, qts):
                    def go():
                        stage_box[bank] = emit_normalize(pp, pch, boxes[bank], qts)
                    return go

                def tp(bank):
                    def go():
                        for qt, stage in stage_box[bank]:
                            emit_transpose(pp, pch, qt, stage)
                    return go

                # The normalize item carries an inflated cost so the pop
                # loop breaks between it and the transposes — the next score
                # matmul then fills the PE pipeline while the DVE muls run
                # (the transposes read their output, so emitting them
                # back-to-back would head-of-line block PE on DVE latency).
                fq_push(0, alloc("X"))
                for g, (qt, hp) in enumerate(((0, 0), (0, 1), (1, 0), (1, 1))):
                    fq_push(8 * 65 * 0.42, av_group("X", g, qt, hp))
                fq_push(STEP_BUDGET, norm("X", (0, 1)))
                fq_push(110, tp("X"))
                for fn in post_x:
                    fn()
                fq_push(0, alloc("Y"))
                for g, (qt, hp) in enumerate(((2, 0), (2, 1), (3, 0), (3, 1))):
                    fq_push(8 * 65 * 0.42, av_group("Y", g, qt, hp))
                fq_push(STEP_BUDGET, norm("Y", (2, 3)))
                fq_push(110, tp("Y"))
                for fn in post_y:
                    fn()

            def enqueue_av_tail(prev, finals):
                """Tail variant of enqueue_av: per-qt stagger so each output
                row's final (j=2 + add + DMA) fires as soon as that qt's two
                AV groups, normalize, and transpose land — instead of
                waiting for a whole bank of four."""
                pp, pch, ppts = prev
                boxes = {}

                def alloc(bank):
                    def go():
                        boxes[bank] = ps_acc.tile(
                            [128, 512], F32, tag="acc", name=f"acct{bank}_{pp}_{pch}"
                        )
                    return go

                def av_group(bank, g, qt, hp):
                    def go():
                        acc = boxes[bank]
                        h = 2 * pp + hp
                        for i in range(8):
                            nc.tensor.matmul(
                                acc[:, g * 128:g * 128 + 65],
                                ppts[i][:, hp * 512 + qt * 128:hp * 512 + (qt + 1) * 128],
                                v_sb[:, i, h * 65:h * 65 + 65],
                                start=(i == 0),
                                stop=(i == 7),
                            )
                    return go

                stage_box = {}

                def norm_qt(bank, qt, qi):
                    def go():
                        stage_box[qt] = emit_normalize_qt(pp, pch, boxes[bank], qt, qi)
                    return go

                def tp_qt(qt):
                    def go():
                        emit_transpose(pp, pch, qt, stage_box[qt], evict_act=True)
                    return go

                # Software-pipelined: qt+1's AV groups sit between qt's
                # normalize (DVE) and its transpose (PE reads the DVE
                # output), so PE never head-of-line blocks on DVE latency.
                # Bank X holds qt0/qt1's groups, bank Y qt2/qt3's; a bank's
                # second qt starts only after its first stopped (sequential
                # accumulation groups), and the normalize reads survive the
                # later start because PSUM zeroing is lazy.
                plan = []
                for qt in range(4):
                    bank = "X" if qt < 2 else "Y"
                    qi = qt % 2
                    items = []
                    if qi == 0:
                        items.append((0, alloc(bank)))
                    for hp in range(2):
                        items.append((8 * 65 * 0.42, av_group(bank, 2 * qi + hp, qt, hp)))
                    items.append((0, norm_qt(bank, qt, qi)))
                    plan.append(items)
                    plan.append([(60, tp_qt(qt)), (0, finals[qt])])
                # interleave: AV(qt+1) between norm(qt) and tp(qt)
                order = [0, 2, 1, 4, 3, 6, 5, 7]
                for idx in order:
                    for cost, fn in plan[idx]:
                        fq_push(cost, fn)

            def attention_unit(p, ch, prev, extra=()):
                """One (pair, query-chunk) unit: 8 x (scores both heads into
                a 2-bank PSUM tile, one 1024-wide exp).  The previous unit's
                AV/normalize/transposes are enqueued first so they fill this
                unit's PE bubbles (the scores pipeline is ACT-bound).
                `extra` closures are enqueued after the AV items — for work
                that depends on the previous unit's attT transposes."""
                qt_ = 2 * p       # qkT tile of this pair's q
                kt_ = 2 * p + 1   # qkT tile of this pair's k
                pts = []

                def score_exp(i):
                    s = ps_s.tile([128, 1024], F32, tag="s", name=f"s_{p}_{ch}_{i}")
                    for hp in range(2):
                        lo, hi = hp * 64, hp * 64 + 64
                        nc.tensor.matmul(
                            s[:, hp * 512:(hp + 1) * 512],
                            qkT[lo:hi, kt_, i * 128:(i + 1) * 128],
                            qkT[lo:hi, qt_, ch * 512:(ch + 1) * 512],
                            start=True,
                            stop=True,
                        )
                    pt = pt_pool.tile([128, 1024], BF16, tag="pt",
                                      name=f"pt_{p}_{ch}_{i}")
                    nc.scalar.activation(pt, s, mybir.ActivationFunctionType.Exp,
                                         scale=SCALE)
                    pts.append(pt)

                score_exp(0)
                score_exp(1)
                if prev is not None:
                    enqueue_av(prev)
                for fn in extra:
                    fn()
                for i in range(2, 8):
                    score_exp(i)
                    fq_pop(STEP_BUDGET)
                fq_pop(2 * STEP_BUDGET)
                return (p, ch, pts)

            # ---- out-projection ------------------------------------------
            CHUNKS = ((0, 512), (512, 256))
            _osb_cache = {}

            def out_group(i, c):
                """Chunk c of row-block i (rows 0..3 path): 3 j-matmuls +
                DVE evict; c==1 flushes the row's 768-wide DMA."""
                c0, cw = CHUNKS[c]
                if c == 0:
                    osb = outsb.tile([128, 768], F32, tag="osb", name=f"osb_{i}")
                    _osb_cache[i] = osb
                else:
                    osb = _osb_cache.pop(i)

                def evict(ps, i=i, c=c, osb=osb):
                    nc.vector.tensor_copy(osb[:, c0:c0 + cw], ps)
                    if c == 1:
                        nc.sync.dma_start(
                            out=out_d[i * 128:(i + 1) * 128, :], in_=osb)
                chain(
                    lambda j, i=i: attT[:, j, i * 128:(i + 1) * 128],
                    lambda j, c0=c0, cw=cw: wo[:, j, c0:c0 + cw],
                    3, cw, evict, f"o_{i}_{c}",
                )

            def out_partial_chain(i, c):
                """j=0,1 of (row-block i, chunk c) into out_partial; c==1
                flushes the row's partial straight to DRAM (the host adds
                the j=2 remainder from out2)."""
                c0, cw = CHUNKS[c]

                def evict(ps, i=i, c=c):
                    nc.vector.tensor_copy(out_partial[:, i - 4, c0:c0 + cw], ps)
                    if c == 1:
                        nc.sync.dma_start(
                            out=out_d[i * 128:(i + 1) * 128, :],
                            in_=out_partial[:, i - 4, :],
                        )
                chain(
                    lambda j, i=i: attT[:, j, i * 128:(i + 1) * 128],
                    lambda j, c0=c0, cw=cw: wo[:, j, c0:c0 + cw],
                    2, cw, evict, f"pp_{i}_{c}",
                )

            def out_final(i):
                """j=2-only remainder of row-block i -> out2 (host adds it
                onto the j=0,1 partials).  ACT evictions — ACT is idle after
                the last exp while DVE still runs the tail normalizes.
                Emits directly (tail drain context)."""
                osb = outsb.tile([128, 768], F32, tag="osb", name=f"osbf_{i}")
                r0 = (i - 4) * 128
                for c, (c0, cw) in enumerate(CHUNKS):
                    ps = ps_aux.tile([128, cw], F32, tag="aux", name=f"f_{i}_{c}",
                                     padded_shape=[128, 512])
                    nc.tensor.matmul(
                        ps,
                        attT[:, 2, i * 128:(i + 1) * 128],
                        wo[:, 2, c0:c0 + cw],
                        start=True,
                        stop=True,
                    )
                    nc.scalar.copy(osb[:, c0:c0 + cw], ps)
                    if i == 7:
                        # flush per chunk so the kernel-end drain only waits
                        # on the short final transfer
                        nc.sync.dma_start(
                            out=out2_d[r0:r0 + 128, c0:c0 + cw],
                            in_=osb[:, c0:c0 + cw],
                        )
                if i != 7:
                    nc.sync.dma_start(out=out2_d[r0:r0 + 128, :], in_=osb)

            # ---- schedule -------------------------------------------------
            # Pre-weave: the minimum projections unit (0,0) needs — q pair 0
            # chunk 0 and k pair 0 both chunks (scores sweep all key blocks).
            # Alternating eviction engines so neither DVE nor ACT gates the
            # first scores.  q chunk 1 flows through the queue.
            qk_group(0, chs=(0,))
            qk_group(1, evict_on_act=True)
            fq_drain()

            # Weave. Query-chunk-0 units first: once (0,0),(1,0),(2,0) are
            # done, output row-blocks 0..3 are fully determined, so their
            # out-projection (and DMA) overlaps the chunk-1 units.  v, the
            # remaining qk projections, AV bursts, and out chains all flow
            # through the fill queue; a full drain before each unit bounds
            # the backlog and guarantees every unit's inputs (its pair's qkT
            # chains, v for the AV bursts) are emitted before its scores.
            qk_group(0, chs=(1,))
            for i in range(8):
                v_group(i)
            prev = attention_unit(0, 0, None)
            qk_group(2)
            qk_group(3)
            fq_drain()
            prev = attention_unit(1, 0, prev)
            qk_group(4)
            qk_group(5)
            fq_drain()
            prev = attention_unit(2, 0, prev)
            fq_drain()
            # out row-blocks 0..3 depend only on the ch-0 attT columns; the
            # last of those (pair 2) is transposed inside unit (0,1) by its
            # AV items, so the row chains can fill units (0,1) and (1,1) —
            # without them the late units' fill queues run dry and PE both
            # idles and drops to the cold p-state.
            prev = attention_unit(0, 1, prev, extra=[
                lambda i=i, c=c: out_group(i, c)
                for i in (0, 1) for c in (0, 1)
            ])
            fq_drain()
            prev = attention_unit(1, 1, prev, extra=[
                lambda i=i, c=c: out_group(i, c)
                for i in (2, 3) for c in (0, 1)
            ])
            fq_drain()
            # rows 4..7 j=0,1 partials read attT pair-0/1 ch-1; pair 1 is
            # only transposed inside unit (2,1) — enqueue them behind its AV
            # items.  The j=2 finals then ride the tail AV's post hooks so
            # each row's add+DMA fires as soon as its attT columns land.
            prev = attention_unit(2, 1, prev, extra=[
                lambda i=i, c=c: out_partial_chain(i, c)
                for i in (4, 5) for c in (0, 1)
            ])
            for i in (6, 7):
                for c in (0, 1):
                    out_partial_chain(i, c)
            enqueue_av_tail(prev, [lambda i=i: out_final(i) for i in (4, 5, 6, 7)])
            fq_drain()

    nc.compile()
    return nc


_NC_CACHE = {}


def _get_nc():
    if "nc" not in _NC_CACHE:
        _NC_CACHE["nc"] = build_nc()
    return _NC_CACHE["nc"]


def kernel(x, w_qkv, w_out, b_out):
    import ml_dtypes

    def bf16(a):
        return np.ascontiguousarray(
            np.asarray(a, dtype=np.float32).astype(ml_dtypes.bfloat16)
        )

    x = np.asarray(x, dtype=np.float32)
    w_qkv = np.asarray(w_qkv, dtype=np.float32)
    w_out = np.asarray(w_out, dtype=np.float32)
    b_out = np.asarray(b_out, dtype=np.float32)

    nc = _get_nc()
    ones_col = np.ones((128, 64), dtype=np.float32)
    ident = np.eye(128, dtype=np.float32)
    in_maps = []
    for c in range(NCORES):
        b = c // 2
        hs = (c % 2) * HEADS_PER_CORE
        q_cols = w_qkv[:, hs * DH:(hs + 6) * DH]
        k_cols = w_qkv[:, 768 + hs * DH:768 + (hs + 6) * DH]
        # pair-packed: [q_p0 | k_p0 | q_p1 | k_p1 | q_p2 | k_p2], 128 each
        wqk_packed = np.concatenate(
            [blk for p in range(3)
             for blk in (q_cols[:, p * 128:(p + 1) * 128],
                         k_cols[:, p * 128:(p + 1) * 128])],
            axis=1,
        )
        in_maps.append({
            "xT": bf16(x[b].T),
            "w_qk": bf16(wqk_packed),
            "w_v": bf16(w_qkv[:, 1536 + hs * DH:1536 + (hs + 6) * DH]),
            "w_o": bf16(w_out[hs * DH:(hs + 6) * DH, :]),
            "ones_col": bf16(ones_col),
            "ident": bf16(ident),
        })

    res = run_bass_kernel_spmd(nc, in_maps, core_ids=list(range(NCORES))).results

    out = np.empty((4, N, DIM), dtype=np.float32)
    for b in range(4):
        out[b] = res[2 * b]["out"] + res[2 * b + 1]["out"] + b_out
        # rows 512.. left the device as (j01 partials, j2 remainder)
        out[b, N // 2:] += res[2 * b]["out2"] + res[2 * b + 1]["out2"]
    return out


# revision 56
# speedup vs baseline: 1.0272x; 1.0057x over previous
"""Multi-head attention forward (B=4, N=1024, D=768, H=12, dh=64) on 8 TRN2 cores.

Sharding: (batch, head-group) — core c handles batch b = c//2 and heads
hs..hs+5 where hs = (c%2)*6.  Each core computes its 6 heads' contribution
to out[b] = attn(x[b]) @ W_out_rows(for its heads); host sums the two
partials per batch and adds the bias (the "all-reduce after final linear").

All SBUF tensors are bf16 (halves DMA + keeps matmuls at 1 cycle/row for
any moving-dim size); PSUM accumulation stays fp32.

Per-core dataflow:
  qkT  [768,1024] = w_qk^T @ x^T          (d-major q,k — feeds scores;
                                           w_qk cols pair-packed
                                           [q_p0|k_p0|q_p1|k_p1|q_p2|k_p2])
  v    [1024,390] = x @ w_v (+ ones col)  (n-major v — feeds AV)
  S^T  [128k,2*512q]/(pair,i) = k_h @ q_h^T  (keys on partitions; the two
                                           heads' q-chunks side by side in
                                           one 2-bank PSUM tile)
  P^T  = exp(S^T * scale)                 (ONE [128,1024] ACT op per (pair,i):
                                           no max-sub — scores ~ N(0,1))
  o    [128q,65]/(head,qtile) = P_slice^T^T @ [v_h|1]  (q-major AV: full 128
                                           PE rows; col 64 = denominator)
  attn = o[:,0:64] * (1/o[:,64])          (per-partition scalar broadcast on
                                           DVE — no broadcast matmul needed)
  attT = PE-transpose(attn)               (back to d-major for the out proj)
  out  [1024,768] = attT^T @ w_o          (partial; host all-reduce)

PSUM accumulation groups zero a whole 2KB bank on start, so the 8 q-major
AV accumulators of a unit (4 per bank x 2 banks) must run sequentially
within each bank: the unit's AV is a 64-matmul burst executed in the NEXT
unit's window (PE is idle there waiting on the exp pipeline).  All deferred
PE work — AV bursts, transposes, projection / out-projection chain pieces —
flows through a FIFO fill queue popped between score matmuls, keeping the
tensor engine saturated at ~2-matmul granularity so the ACT exp cadence
never starves.
"""
import os
import sys

sys.path.insert(0, "/opt/trn_rl_repo")

# The kernel needs the axon-tunneled TRN2 PJRT backend; a JAX_PLATFORMS=cpu
# pin (common for reference-side jax) would hide the NeuronCores.
if os.environ.get("JAX_PLATFORMS", "").strip() == "cpu":
    del os.environ["JAX_PLATFORMS"]

import numpy as np
import concourse.bass as bass
import concourse.bacc as bacc
import concourse.tile as tile
from concourse import mybir
from concourse.bass_utils import run_bass_kernel_spmd
from contextlib import ExitStack

F32 = mybir.dt.float32
BF16 = mybir.dt.bfloat16

DIM = 768
N = 1024
HEADS_PER_CORE = 6
DH = 64
SCALE = DH ** -0.5
NCORES = 8
N_WARM = int(os.environ.get("ATTN_N_WARM", "32"))
STEP_BUDGET = float(os.environ.get("ATTN_STEP_BUDGET", "640"))


def build_nc():
    DT = BF16
    nc = bacc.Bacc("TRN2", target_bir_lowering=False, debug=False)

    xT_d = nc.declare_dram_parameter("xT", [DIM, N], DT, isOutput=False)
    wqk_d = nc.declare_dram_parameter("w_qk", [DIM, 768], DT, isOutput=False)
    wv_d = nc.declare_dram_parameter("w_v", [DIM, 384], DT, isOutput=False)
    wo_d = nc.declare_dram_parameter("w_o", [384, DIM], DT, isOutput=False)
    ones_d = nc.declare_dram_parameter("ones_col", [128, 64], DT, isOutput=False)
    ident_d = nc.declare_dram_parameter("ident", [128, 128], DT, isOutput=False)
    out_d = nc.declare_dram_parameter("out", [N, DIM], F32, isOutput=True)
    # rows 4..7 leave the device in two pieces summed on the host: j=0,1
    # partials (DMA'd during the last unit) go to out, the j=2 remainder
    # (tail) to out2 — killing the on-device combining adds, which were the
    # serial-DVE bottleneck of the tail.
    out2_d = nc.declare_dram_parameter("out2", [N // 2, DIM], F32, isOutput=True)

    with tile.TileContext(nc) as tc:
        with ExitStack() as ctx:
            persist = ctx.enter_context(tc.tile_pool(name="persist", bufs=1))
            # pt ring sized for ~2.5 units in flight so a unit's AV burst may
            # spill into the unit after next without blocking the exp pipeline
            pt_pool = ctx.enter_context(tc.tile_pool(name="pt", bufs=20))
            attn_pool = ctx.enter_context(tc.tile_pool(name="attn", bufs=6))
            stats = ctx.enter_context(tc.tile_pool(name="stats", bufs=4))
            outsb = ctx.enter_context(tc.tile_pool(name="outsb", bufs=4))
            # PSUM slots are bank-granular: 2 x 2-bank score tiles + 2 x
            # 1-bank AV accumulators + 2 x 1-bank aux slots = all 8 banks.
            ps_s = ctx.enter_context(tc.tile_pool(name="ps_s", bufs=2, space="PSUM"))
            ps_acc = ctx.enter_context(tc.tile_pool(name="ps_acc", bufs=2, space="PSUM"))
            ps_aux = ctx.enter_context(tc.tile_pool(name="ps_aux", bufs=2, space="PSUM"))

            xT = persist.tile([128, 6, N], DT)
            wqk = persist.tile([128, 6, 768], DT)
            wv = persist.tile([128, 6, 384], DT)
            wo = persist.tile([128, 3, 768], DT)
            qkT = persist.tile([128, 6, N], DT)
            v_sb = persist.tile([128, 8, 6 * 65], DT)
            attT = persist.tile([128, 3, N], DT)
            ident = persist.tile([128, 128], DT)
            out_partial = persist.tile([128, 4, DIM], F32)
            warm_src = persist.tile([128, 256], DT)

            # Input DMAs, one batched transfer per tensor (each dma_start
            # costs ~0.6us on the shared HWDGE generator + ~0.9us sem
            # propagation, so fewer/bigger is strictly better).  Transfer
            # order on the exclusive DMA device: wqk pair-0 cols + xT first
            # (they gate the first projection chains), then w_v, the rest of
            # w_qk, w_o.  Two queues (SP + ACT) halve issue latency.
            # All input DMAs on the SP queue in strict priority order — the
            # DMA device is exclusive, so a second queue would only let a
            # low-priority transfer cut ahead of the chain-gating wqk/xT
            # pair (issue costs pipeline ahead of the transfers anyway).
            nc.sync.dma_start(
                out=wqk[:, :, 0:256],
                in_=wqk_d[:, 0:256].rearrange("(k p) n -> p k n", p=128),
            )
            nc.sync.dma_start(
                out=xT, in_=xT_d.rearrange("(k p) n -> p k n", p=128)
            )
            nc.sync.dma_start(
                out=wv, in_=wv_d.rearrange("(k p) n -> p k n", p=128)
            )
            nc.sync.dma_start(
                out=wqk[:, :, 256:768],
                in_=wqk_d[:, 256:768].rearrange("(k p) n -> p k n", p=128),
            )
            nc.sync.dma_start(
                out=wo, in_=wo_d.rearrange("(k p) n -> p k n", p=128)
            )
            ones_stage = persist.tile([128, 64], DT)
            nc.sync.dma_start(out=ones_stage, in_=ones_d[:, :])
            nc.sync.dma_start(out=ident, in_=ident_d[:, :])
            # ones: v_sb[:, i, h*65 + 64] = 1.0 for all (i, h). The strided
            # scatter runs on the otherwise-idle GPSIMD (SBUF->SBUF is its
            # one legal niche) so it can never head-of-line block DVE's
            # projection evictions; as a DMA it would cost 6144 min-size
            # descriptors.
            v_ones_view = v_sb.rearrange("p i (h c) -> p i h c", h=6)[:, :, :, 64]
            nc.gpsimd.tensor_copy(
                v_ones_view, ones_stage[:, 0:48].rearrange("p (i h) -> p i h", i=8)
            )

            # PE clock warm-up: the tensor engine idles during the input DMA
            # window and would otherwise start the projection at the cold
            # p-state (and the ramp needs 3us of continuous execution to
            # reach full clock).  Matmuls against a GPSIMD-memset SBUF tile
            # keep PE busy across the DMA window with no data dependencies;
            # the trailing N=1 ones keep the tail cheap so the first real
            # chain isn't delayed.
            nc.gpsimd.memset(warm_src, 1.0)
            # Preload the ACT exp table during the DMA window (first real exp
            # would otherwise pay the ~1.3us table load at weave start).
            warm_exp = stats.tile([128, 1], F32, tag="warm_exp", name="warm_exp")
            nc.scalar.activation(warm_exp, warm_src[:, 0:1],
                                 mybir.ActivationFunctionType.Exp, scale=SCALE)
            warm_ps = ps_aux.tile([1, 256], F32, tag="aux", name="warm_ps",
                                  padded_shape=[128, 512])
            for _w in range(N_WARM):
                nc.tensor.matmul(warm_ps, warm_src[:, 0:1], warm_src,
                                 start=True, stop=True)
            for _w in range(8):
                nc.tensor.matmul(warm_ps[:, 0:1], warm_src[:, 0:1],
                                 warm_src[:, 0:1], start=True, stop=True)

            # ---- deferred-PE-work fill queue ------------------------------
            # (cost_ns, closure) FIFO; attention units pop ~STEP_BUDGET ns of
            # work between score matmuls.  Emission order == pop order, so
            # enqueue order must respect data deps.
            fill_q = []

            def fq_push(cost, fn):
                fill_q.append((cost, fn))

            def fq_pop(budget):
                spent = 0.0
                while fill_q and spent < budget:
                    cost, fn = fill_q.pop(0)
                    fn()
                    spent += cost

            def fq_drain():
                while fill_q:
                    fill_q.pop(0)[1]()

            # ---- projection chains (as fine-grained queue items) ---------
            def chain(lhs_fn, rhs_fn, n_k, width, evict_fn, name):
                """Accumulating matmul chain split into 2-matmul queue items
                + an eviction item.  lhs_fn/rhs_fn map kt -> AP."""
                box = {}

                def piece(k0, k1):
                    def go():
                        if k0 == 0:
                            box["ps"] = ps_aux.tile(
                                [128, width], F32, tag="aux", name=name,
                                padded_shape=[128, 512])
                        ps = box["ps"]
                        for kt in range(k0, k1):
                            nc.tensor.matmul(
                                ps, lhs_fn(kt), rhs_fn(kt),
                                start=(kt == 0), stop=(kt == n_k - 1),
                            )
                    return go

                for k0 in range(0, n_k, 2):
                    k1 = min(k0 + 2, n_k)
                    fq_push((k1 - k0) * width * 0.42, piece(k0, k1))
                fq_push(0, lambda: evict_fn(box["ps"]))

            def qk_group(mt, chs=(0, 1), evict_on_act=False):
                """qkT[mt, ch-chunk] = (w_qk col-block mt)^T @ xT.
                Col blocks (pair-packed): mt=2p -> q of pair p, 2p+1 -> k.
                evict_on_act alternates the two chunks across ACT/DVE so the
                evictions (which gate the first scores) run in parallel."""
                for ch in chs:
                    def evict(ps, mt=mt, ch=ch):
                        dst = qkT[:, mt, ch * 512:(ch + 1) * 512]
                        if evict_on_act and ch == 0:
                            nc.scalar.copy(dst, ps)
                        else:
                            nc.vector.tensor_copy(dst, ps)
                    chain(
                        lambda kt, mt=mt: wqk[:, kt, mt * 128:(mt + 1) * 128],
                        lambda kt, ch=ch: xT[:, kt, ch * 512:(ch + 1) * 512],
                        6, 512, evict, f"qk_{mt}_{ch}",
                    )

            def v_group(i):
                """v rows-block i = x[i-block] @ w_v, strided into v_sb."""
                def evict(ps, i=i):
                    dst = v_sb[:, i, :].rearrange("p (h c) -> p h c", h=6)[:, :, 0:DH]
                    nc.vector.tensor_copy(dst, ps.rearrange("p (h c) -> p h c", h=6))
                chain(
                    lambda kt, i=i: xT[:, kt, i * 128:(i + 1) * 128],
                    lambda kt: wv[:, kt, :],
                    6, 384, evict, f"v_{i}",
                )

            # ---- attention ------------------------------------------------
            def emit_normalize_qt(p, ch, acc, qt, qi):
                """Single-qt normalize: groups (qi*2, qi*2+1) of `acc`."""
                dinv = stats.tile([128, 2], F32, tag="dinv",
                                  name=f"dinvq_{p}_{ch}_{qt}")
                dview = acc.rearrange("p (g c) -> p g c", g=4)[:, 2 * qi:2 * qi + 2, 64]
                with nc.allow_low_precision(
                    reason="softmax denominators are O(100); rounding of "
                    "1/denom is below the bf16 noise floor of the weave"
                ):
                    nc.vector.reciprocal(dinv, dview)
                stage = attn_pool.tile(
                    [128, 128], BF16, tag="attn", name=f"attnq_{p}_{ch}_{qt}"
                )
                for hp in range(2):
                    nc.vector.tensor_scalar_mul(
                        stage[:, hp * 64:hp * 64 + 64],
                        acc[:, (2 * qi + hp) * 128:(2 * qi + hp) * 128 + 64],
                        dinv[:, hp:hp + 1],
                    )
                return stage

            def emit_normalize(p, ch, acc, qts):
                """acc holds 4 groups (qt, hp) at 128-col stride, col 64 of
                each group = softmax denominator.  DVE: one strided
                reciprocal + 4 per-partition-scalar muls into bf16 staging."""
                dinv = stats.tile([128, 4], F32, tag="dinv",
                                  name=f"dinv_{p}_{ch}_{qts[0]}")
                dview = acc.rearrange("p (g c) -> p g c", g=4)[:, :, 64]
                with nc.allow_low_precision(
                    reason="softmax denominators are O(100); rounding of "
                    "1/denom is below the bf16 noise floor of the weave"
                ):
                    nc.vector.reciprocal(dinv, dview)
                stages = []
                for qi, qt in enumerate(qts):
                    stage = attn_pool.tile(
                        [128, 128], BF16, tag="attn", name=f"attn_{p}_{ch}_{qt}"
                    )
                    for hp in range(2):
                        g = qi * 2 + hp
                        nc.vector.tensor_scalar_mul(
                            stage[:, hp * 64:hp * 64 + 64],
                            acc[:, g * 128:g * 128 + 64],
                            dinv[:, g:g + 1],
                        )
                    stages.append((qt, stage))
                return stages

            def emit_transpose(p, ch, qt, stage, evict_act=False):
                """PE-transpose one [128q, 128d(2 heads)] staging tile into
                d-major attT columns; bf16 PSUM via bitcast.  Eviction on DVE
                (weave) or ACT (tail, where ACT is idle and DVE is the
                serial bottleneck)."""
                tp_f32 = ps_aux.tile([128, 64], F32, tag="aux",
                                     name=f"tp_{p}_{ch}_{qt}",
                                     padded_shape=[128, 512])
                tp = tp_f32.bitcast(BF16)
                nc.tensor.matmul(tp, stage, ident, is_transpose=True,
                                 start=True, stop=True)
                dst = attT[:, p, ch * 512 + qt * 128:ch * 512 + (qt + 1) * 128]
                if evict_act:
                    nc.scalar.copy(dst, tp)
                else:
                    nc.vector.tensor_copy(dst, tp)

            def enqueue_av(prev, post_x=(), post_y=()):
                """Enqueue the AV burst + normalize + transposes for a
                finished unit.  PSUM groups zero a whole bank on start, so
                the 4 (qt,hp) groups of each bank run sequentially (each
                fully accumulated over i=0..7 before the next starts).
                `post_x`/`post_y` callbacks push follow-up work right behind
                each bank's normalize+transposes (used by the tail so each
                output row's final fires as soon as its attT columns land)."""
                pp, pch, ppts = prev
                boxes = {}

                def alloc(bank):
                    def go():
                        boxes[bank] = ps_acc.tile(
                            [128, 512], F32, tag="acc", name=f"acc{bank}_{pp}_{pch}"
                        )
                    return go

                def av_group(bank, g, qt, hp):
                    def go():
                        acc = boxes[bank]
                        h = 2 * pp + hp
                        for i in range(8):
                            nc.tensor.matmul(
                                acc[:, g * 128:g * 128 + 65],
                                ppts[i][:, hp * 512 + qt * 128:hp * 512 + (qt + 1) * 128],
                                v_sb[:, i, h * 65:h * 65 + 65],
                                start=(i == 0),
                                stop=(i == 7),
                            )
                    return go

                stage_box = {}

                def norm(bank, qts):
                    def go():
                        stage_box[bank] = emit_normalize(pp, pch, boxes[bank], qts)
                    return go

                def tp(bank):
                    def go():
                        for qt, stage in stage_box[bank]:
                            emit_transpose(pp, pch, qt, stage)
                    return go

                # The normalize item carries an inflated cost so the pop
                # loop breaks between it and the transposes — the next score
                # matmul then fills the PE pipeline while the DVE muls run
                # (the transposes read their output, so emitting them
                # back-to-back would head-of-line block PE on DVE latency).
                fq_push(0, alloc("X"))
                for g, (qt, hp) in enumerate(((0, 0), (0, 1), (1, 0), (1, 1))):
                    fq_push(8 * 65 * 0.42, av_group("X", g, qt, hp))
                fq_push(STEP_BUDGET, norm("X", (0, 1)))
                fq_push(110, tp("X"))
                for fn in post_x:
                    fn()
                fq_push(0, alloc("Y"))
                for g, (qt, hp) in enumerate(((2, 0), (2, 1), (3, 0), (3, 1))):
                    fq_push(8 * 65 * 0.42, av_group("Y", g, qt, hp))
                fq_push(STEP_BUDGET, norm("Y", (2, 3)))
                fq_push(110, tp("Y"))
                for fn in post_y:
                    fn()

            def enqueue_av_tail(prev, finals, pre=()):
                """Tail variant of enqueue_av: per-qt stagger so each output
                row's final (j=2 + add + DMA) fires as soon as that qt's two
                AV groups, normalize, and transpose land — instead of
                waiting for a whole bank of four."""
                pp, pch, ppts = prev
                boxes = {}

                def alloc(bank):
                    def go():
                        boxes[bank] = ps_acc.tile(
                            [128, 512], F32, tag="acc", name=f"acct{bank}_{pp}_{pch}"
                        )
                    return go

                def av_group(bank, g, qt, hp):
                    def go():
                        acc = boxes[bank]
                        h = 2 * pp + hp
                        for i in range(8):
                            nc.tensor.matmul(
                                acc[:, g * 128:g * 128 + 65],
                                ppts[i][:, hp * 512 + qt * 128:hp * 512 + (qt + 1) * 128],
                                v_sb[:, i, h * 65:h * 65 + 65],
                                start=(i == 0),
                                stop=(i == 7),
                            )
                    return go

                stage_box = {}

                def norm_qt(bank, qt, qi):
                    def go():
                        stage_box[qt] = emit_normalize_qt(pp, pch, boxes[bank], qt, qi)
                    return go

                def tp_qt(qt):
                    def go():
                        emit_transpose(pp, pch, qt, stage_box[qt],
                                       evict_act=(qt % 2 == 0))
                    return go

                # All AV bursts + normalizes first (PE work with no
                # cross-engine stalls; the DVE normalizes overlap), then the
                # per-row transpose -> final -> DMA chains, evictions
                # alternating ACT/DVE so neither engine serializes the tail.
                # Bank X holds qt0/qt1's groups, bank Y qt2/qt3's; a bank's
                # second qt starts only after its first stopped (sequential
                # accumulation groups), and the normalize reads survive the
                # later start because PSUM zeroing is lazy.
                for qt in range(4):
                    bank = "X" if qt < 2 else "Y"
                    qi = qt % 2
                    if qi == 0:
                        fq_push(0, alloc(bank))
                    for hp in range(2):
                        fq_push(8 * 65 * 0.42, av_group(bank, 2 * qi + hp, qt, hp))
                    fq_push(0, norm_qt(bank, qt, qi))
                    if qt == 0:
                        # independent leftovers slot in behind the first AV
                        # burst, covering the exp-drain window
                        for fn in pre:
                            fn()
                for qt in range(4):
                    fq_push(60, tp_qt(qt))
                    fq_push(0, finals[qt])

            def attention_unit(p, ch, prev, extra=()):
                """One (pair, query-chunk) unit: 8 x (scores both heads into
                a 2-bank PSUM tile, one 1024-wide exp).  The previous unit's
                AV/normalize/transposes are enqueued first so they fill this
                unit's PE bubbles (the scores pipeline is ACT-bound).
                `extra` closures are enqueued after the AV items — for work
                that depends on the previous unit's attT transposes."""
                qt_ = 2 * p       # qkT tile of this pair's q
                kt_ = 2 * p + 1   # qkT tile of this pair's k
                pts = []

                def score_exp(i):
                    s = ps_s.tile([128, 1024], F32, tag="s", name=f"s_{p}_{ch}_{i}")
                    for hp in range(2):
                        lo, hi = hp * 64, hp * 64 + 64
                        nc.tensor.matmul(
                            s[:, hp * 512:(hp + 1) * 512],
                            qkT[lo:hi, kt_, i * 128:(i + 1) * 128],
                            qkT[lo:hi, qt_, ch * 512:(ch + 1) * 512],
                            start=True,
                            stop=True,
                        )
                    pt = pt_pool.tile([128, 1024], BF16, tag="pt",
                                      name=f"pt_{p}_{ch}_{i}")
                    nc.scalar.activation(pt, s, mybir.ActivationFunctionType.Exp,
                                         scale=SCALE)
                    pts.append(pt)

                score_exp(0)
                score_exp(1)
                if prev is not None:
                    enqueue_av(prev)
                for fn in extra:
                    fn()
                for i in range(2, 8):
                    score_exp(i)
                    fq_pop(STEP_BUDGET)
                fq_pop(2 * STEP_BUDGET)
                return (p, ch, pts)

            # ---- out-projection ------------------------------------------
            CHUNKS = ((0, 512), (512, 256))
            _osb_cache = {}

            def out_group(i, c):
                """Chunk c of row-block i (rows 0..3 path): 3 j-matmuls +
                DVE evict; c==1 flushes the row's 768-wide DMA."""
                c0, cw = CHUNKS[c]
                if c == 0:
                    osb = outsb.tile([128, 768], F32, tag="osb", name=f"osb_{i}")
                    _osb_cache[i] = osb
                else:
                    osb = _osb_cache.pop(i)

                def evict(ps, i=i, c=c, osb=osb):
                    nc.vector.tensor_copy(osb[:, c0:c0 + cw], ps)
                    if c == 1:
                        nc.sync.dma_start(
                            out=out_d[i * 128:(i + 1) * 128, :], in_=osb)
                chain(
                    lambda j, i=i: attT[:, j, i * 128:(i + 1) * 128],
                    lambda j, c0=c0, cw=cw: wo[:, j, c0:c0 + cw],
                    3, cw, evict, f"o_{i}_{c}",
                )

            def out_partial_chain(i, c):
                """j=0,1 of (row-block i, chunk c) into out_partial; c==1
                flushes the row's partial straight to DRAM (the host adds
                the j=2 remainder from out2)."""
                c0, cw = CHUNKS[c]

                def evict(ps, i=i, c=c):
                    nc.vector.tensor_copy(out_partial[:, i - 4, c0:c0 + cw], ps)
                    if c == 1:
                        nc.sync.dma_start(
                            out=out_d[i * 128:(i + 1) * 128, :],
                            in_=out_partial[:, i - 4, :],
                        )
                chain(
                    lambda j, i=i: attT[:, j, i * 128:(i + 1) * 128],
                    lambda j, c0=c0, cw=cw: wo[:, j, c0:c0 + cw],
                    2, cw, evict, f"pp_{i}_{c}",
                )

            def out_final(i):
                """j=2-only remainder of row-block i -> out2 (host adds it
                onto the j=0,1 partials).  Evictions alternate ACT/DVE by
                row so neither engine serializes the tail.  Emits directly
                (tail drain context)."""
                osb = outsb.tile([128, 768], F32, tag="osb", name=f"osbf_{i}")
                r0 = (i - 4) * 128
                for c, (c0, cw) in enumerate(CHUNKS):
                    ps = ps_aux.tile([128, cw], F32, tag="aux", name=f"f_{i}_{c}",
                                     padded_shape=[128, 512])
                    nc.tensor.matmul(
                        ps,
                        attT[:, 2, i * 128:(i + 1) * 128],
                        wo[:, 2, c0:c0 + cw],
                        start=True,
                        stop=True,
                    )
                    if i % 2 == 0:
                        nc.scalar.copy(osb[:, c0:c0 + cw], ps)
                    else:
                        nc.vector.tensor_copy(osb[:, c0:c0 + cw], ps)
                    if i == 7:
                        # flush per chunk so the kernel-end drain only waits
                        # on the short final transfer
                        nc.sync.dma_start(
                            out=out2_d[r0:r0 + 128, c0:c0 + cw],
                            in_=osb[:, c0:c0 + cw],
                        )
                if i != 7:
                    nc.sync.dma_start(out=out2_d[r0:r0 + 128, :], in_=osb)

            # ---- schedule -------------------------------------------------
            # Pre-weave: the minimum projections unit (0,0) needs — q pair 0
            # chunk 0 and k pair 0 both chunks (scores sweep all key blocks).
            # Alternating eviction engines so neither DVE nor ACT gates the
            # first scores.  q chunk 1 flows through the queue.
            qk_group(0, chs=(0,))
            qk_group(1, evict_on_act=True)
            fq_drain()

            # Weave. Query-chunk-0 units first: once (0,0),(1,0),(2,0) are
            # done, output row-blocks 0..3 are fully determined, so their
            # out-projection (and DMA) overlaps the chunk-1 units.  v, the
            # remaining qk projections, AV bursts, and out chains all flow
            # through the fill queue; a full drain before each unit bounds
            # the backlog and guarantees every unit's inputs (its pair's qkT
            # chains, v for the AV bursts) are emitted before its scores.
            qk_group(0, chs=(1,))
            for i in range(8):
                v_group(i)
            prev = attention_unit(0, 0, None)
            qk_group(2)
            qk_group(3)
            fq_drain()
            prev = attention_unit(1, 0, prev)
            qk_group(4)
            qk_group(5)
            fq_drain()
            prev = attention_unit(2, 0, prev)
            fq_drain()
            # out row-blocks 0..3 depend only on the ch-0 attT columns; the
            # last of those (pair 2) is transposed inside unit (0,1) by its
            # AV items, so the row chains can fill units (0,1) and (1,1) —
            # without them the late units' fill queues run dry and PE both
            # idles and drops to the cold p-state.
            prev = attention_unit(0, 1, prev, extra=[
                lambda i=i, c=c: out_group(i, c)
                for i in (0, 1) for c in (0, 1)
            ])
            fq_drain()
            prev = attention_unit(1, 1, prev, extra=[
                lambda i=i, c=c: out_group(i, c)
                for i in (2, 3) for c in (0, 1)
            ])
            fq_drain()
            # rows 4..7 j=0,1 partials read attT pair-0/1 ch-1; pair 1 is
            # only transposed inside unit (2,1) — enqueue them behind its AV
            # items.  The j=2 finals then ride the tail AV's post hooks so
            # each row's add+DMA fires as soon as its attT columns land.
            prev = attention_unit(2, 1, prev, extra=[
                lambda i=i, c=c: out_partial_chain(i, c)
                for i in (4, 5) for c in (0, 1)
            ])
            out_partial_chain(6, 0)
            out_partial_chain(6, 1)
            enqueue_av_tail(prev, [lambda i=i: out_final(i) for i in (4, 5, 6, 7)],
                            pre=[lambda: out_partial_chain(7, 0),
                                 lambda: out_partial_chain(7, 1)])
            fq_drain()

    nc.compile()
    return nc


_NC_CACHE = {}


def _get_nc():
    if "nc" not in _NC_CACHE:
        _NC_CACHE["nc"] = build_nc()
    return _NC_CACHE["nc"]


def kernel(x, w_qkv, w_out, b_out):
    import ml_dtypes

    def bf16(a):
        return np.ascontiguousarray(
            np.asarray(a, dtype=np.float32).astype(ml_dtypes.bfloat16)
        )

    x = np.asarray(x, dtype=np.float32)
    w_qkv = np.asarray(w_qkv, dtype=np.float32)
    w_out = np.asarray(w_out, dtype=np.float32)
    b_out = np.asarray(b_out, dtype=np.float32)

    nc = _get_nc()
    ones_col = np.ones((128, 64), dtype=np.float32)
    ident = np.eye(128, dtype=np.float32)
    in_maps = []
    for c in range(NCORES):
        b = c // 2
        hs = (c % 2) * HEADS_PER_CORE
        q_cols = w_qkv[:, hs * DH:(hs + 6) * DH]
        k_cols = w_qkv[:, 768 + hs * DH:768 + (hs + 6) * DH]
        # pair-packed: [q_p0 | k_p0 | q_p1 | k_p1 | q_p2 | k_p2], 128 each
        wqk_packed = np.concatenate(
            [blk for p in range(3)
             for blk in (q_cols[:, p * 128:(p + 1) * 128],
                         k_cols[:, p * 128:(p + 1) * 128])],
            axis=1,
        )
        in_maps.append({
            "xT": bf16(x[b].T),
            "w_qk": bf16(wqk_packed),
            "w_v": bf16(w_qkv[:, 1536 + hs * DH:1536 + (hs + 6) * DH]),
            "w_o": bf16(w_out[hs * DH:(hs + 6) * DH, :]),
            "ones_col": bf16(ones_col),
            "ident": bf16(ident),
        })

    res = run_bass_kernel_spmd(nc, in_maps, core_ids=list(range(NCORES))).results

    out = np.empty((4, N, DIM), dtype=np.float32)
    for b in range(4):
        out[b] = res[2 * b]["out"] + res[2 * b + 1]["out"] + b_out
        # rows 512.. left the device as (j01 partials, j2 remainder)
        out[b, N // 2:] += res[2 * b]["out2"] + res[2 * b + 1]["out2"]
    return out


# revision 63
# speedup vs baseline: 1.0681x; 1.0398x over previous
"""Multi-head attention forward (B=4, N=1024, D=768, H=12, dh=64) on 8 TRN2 cores.

Sharding: (batch, head-group) — core c handles batch b = c//2 and heads
hs..hs+5 where hs = (c%2)*6.  Each core computes its 6 heads' contribution
to out[b] = attn(x[b]) @ W_out_rows(for its heads); host sums the two
partials per batch and adds the bias (the "all-reduce after final linear").

All SBUF tensors are bf16 (halves DMA + keeps matmuls at 1 cycle/row for
any moving-dim size); PSUM accumulation stays fp32.

Per-core dataflow:
  qkT  [768,1024] = w_qk^T @ x^T          (d-major q,k — feeds scores;
                                           w_qk cols pair-packed
                                           [q_p0|k_p0|q_p1|k_p1|q_p2|k_p2])
  v    [1024,390] = x @ w_v (+ ones col)  (n-major v — feeds AV)
  S^T  [128k,2*512q]/(pair,i) = k_h @ q_h^T  (keys on partitions; the two
                                           heads' q-chunks side by side in
                                           one 2-bank PSUM tile)
  P^T  = exp(S^T * scale)                 (ONE [128,1024] ACT op per (pair,i):
                                           no max-sub — scores ~ N(0,1))
  o    [128q,65]/(head,qtile) = P_slice^T^T @ [v_h|1]  (q-major AV: full 128
                                           PE rows; col 64 = denominator)
  attn = o[:,0:64] * (1/o[:,64])          (per-partition scalar broadcast on
                                           DVE — no broadcast matmul needed)
  attT = PE-transpose(attn)               (back to d-major for the out proj)
  out  [1024,768] = attT^T @ w_o          (partial; host all-reduce)

PSUM accumulation groups zero a whole 2KB bank on start, so the 8 q-major
AV accumulators of a unit (4 per bank x 2 banks) must run sequentially
within each bank: the unit's AV is a 64-matmul burst executed in the NEXT
unit's window (PE is idle there waiting on the exp pipeline).  All deferred
PE work — AV bursts, transposes, projection / out-projection chain pieces —
flows through a FIFO fill queue popped between score matmuls, keeping the
tensor engine saturated at ~2-matmul granularity so the ACT exp cadence
never starves.
"""
import os
import sys

sys.path.insert(0, "/opt/trn_rl_repo")

# The kernel needs the axon-tunneled TRN2 PJRT backend; a JAX_PLATFORMS=cpu
# pin (common for reference-side jax) would hide the NeuronCores.
if os.environ.get("JAX_PLATFORMS", "").strip() == "cpu":
    del os.environ["JAX_PLATFORMS"]

import numpy as np
import concourse.bass as bass
import concourse.bacc as bacc
import concourse.tile as tile
from concourse import mybir
from concourse.bass_utils import run_bass_kernel_spmd
from contextlib import ExitStack

F32 = mybir.dt.float32
BF16 = mybir.dt.bfloat16

DIM = 768
N = 1024
HEADS_PER_CORE = 6
DH = 64
SCALE = DH ** -0.5
NCORES = 8
N_WARM = int(os.environ.get("ATTN_N_WARM", "32"))
STEP_BUDGET = float(os.environ.get("ATTN_STEP_BUDGET", "640"))


def build_nc():
    DT = BF16
    nc = bacc.Bacc("TRN2", target_bir_lowering=False, debug=False)

    xT_d = nc.declare_dram_parameter("xT", [DIM, N], DT, isOutput=False)
    wqk_d = nc.declare_dram_parameter("w_qk", [DIM, 768], DT, isOutput=False)
    wv_d = nc.declare_dram_parameter("w_v", [DIM, 384], DT, isOutput=False)
    wo_d = nc.declare_dram_parameter("w_o", [384, DIM], DT, isOutput=False)
    ones_d = nc.declare_dram_parameter("ones_col", [128, 64], DT, isOutput=False)
    ident_d = nc.declare_dram_parameter("ident", [128, 128], DT, isOutput=False)
    out_d = nc.declare_dram_parameter("out", [N, DIM], F32, isOutput=True)
    # rows 4..7 leave the device in two pieces summed on the host: j=0,1
    # partials (DMA'd during the last unit) go to out, the j=2 remainder
    # (tail) to out2 — killing the on-device combining adds, which were the
    # serial-DVE bottleneck of the tail.
    out2_d = nc.declare_dram_parameter("out2", [N // 2, DIM], F32, isOutput=True)

    with tile.TileContext(nc) as tc:
        with ExitStack() as ctx:
            persist = ctx.enter_context(tc.tile_pool(name="persist", bufs=1))
            # pt ring sized for ~3 units in flight so a unit's AV burst may
            # spill two units ahead without blocking the exp pipeline
            pt_pool = ctx.enter_context(tc.tile_pool(name="pt", bufs=26))
            attn_pool = ctx.enter_context(tc.tile_pool(name="attn", bufs=6))
            stats = ctx.enter_context(tc.tile_pool(name="stats", bufs=4))
            outsb = ctx.enter_context(tc.tile_pool(name="outsb", bufs=4))
            # PSUM slots are bank-granular: 2 x 2-bank score tiles + 2 x
            # 1-bank AV accumulators + 2 x 1-bank aux slots = all 8 banks.
            ps_s = ctx.enter_context(tc.tile_pool(name="ps_s", bufs=2, space="PSUM"))
            ps_acc = ctx.enter_context(tc.tile_pool(name="ps_acc", bufs=2, space="PSUM"))
            ps_aux = ctx.enter_context(tc.tile_pool(name="ps_aux", bufs=2, space="PSUM"))

            xT = persist.tile([128, 6, N], DT)
            wqk = persist.tile([128, 6, 768], DT)
            wv = persist.tile([128, 6, 384], DT)
            wo = persist.tile([128, 3, 768], DT)
            qkT = persist.tile([128, 6, N], DT)
            v_sb = persist.tile([128, 8, 6 * 65], DT)
            attT = persist.tile([128, 3, N], DT)
            ident = persist.tile([128, 128], DT)
            out_partial = persist.tile([128, 4, DIM], F32)
            warm_src = persist.tile([128, 256], DT)

            # Input DMAs, one batched transfer per tensor (each dma_start
            # costs ~0.6us on the shared HWDGE generator + ~0.9us sem
            # propagation, so fewer/bigger is strictly better).  Transfer
            # order on the exclusive DMA device: wqk pair-0 cols + xT first
            # (they gate the first projection chains), then w_v, the rest of
            # w_qk, w_o.  Two queues (SP + ACT) halve issue latency.
            # All input DMAs on the SP queue in strict priority order — the
            # DMA device is exclusive, so a second queue would only let a
            # low-priority transfer cut ahead of the chain-gating wqk/xT
            # pair (issue costs pipeline ahead of the transfers anyway).
            nc.sync.dma_start(
                out=wqk[:, :, 0:256],
                in_=wqk_d[:, 0:256].rearrange("(k p) n -> p k n", p=128),
            )
            nc.sync.dma_start(
                out=xT, in_=xT_d.rearrange("(k p) n -> p k n", p=128)
            )
            nc.sync.dma_start(
                out=wv, in_=wv_d.rearrange("(k p) n -> p k n", p=128)
            )
            nc.sync.dma_start(
                out=wqk[:, :, 256:768],
                in_=wqk_d[:, 256:768].rearrange("(k p) n -> p k n", p=128),
            )
            nc.sync.dma_start(
                out=wo, in_=wo_d.rearrange("(k p) n -> p k n", p=128)
            )
            ones_stage = persist.tile([128, 64], DT)
            nc.sync.dma_start(out=ones_stage, in_=ones_d[:, :])
            nc.sync.dma_start(out=ident, in_=ident_d[:, :])
            # ones: v_sb[:, i, h*65 + 64] = 1.0 for all (i, h). The strided
            # scatter runs on the otherwise-idle GPSIMD (SBUF->SBUF is its
            # one legal niche) so it can never head-of-line block DVE's
            # projection evictions; as a DMA it would cost 6144 min-size
            # descriptors.
            v_ones_view = v_sb.rearrange("p i (h c) -> p i h c", h=6)[:, :, :, 64]
            nc.gpsimd.tensor_copy(
                v_ones_view, ones_stage[:, 0:48].rearrange("p (i h) -> p i h", i=8)
            )

            # PE clock warm-up: the tensor engine idles during the input DMA
            # window and would otherwise start the projection at the cold
            # p-state (and the ramp needs 3us of continuous execution to
            # reach full clock).  Matmuls against a GPSIMD-memset SBUF tile
            # keep PE busy across the DMA window with no data dependencies;
            # the trailing N=1 ones keep the tail cheap so the first real
            # chain isn't delayed.
            nc.gpsimd.memset(warm_src, 1.0)
            # Preload the ACT exp table during the DMA window (first real exp
            # would otherwise pay the ~1.3us table load at weave start).
            warm_exp = stats.tile([128, 1], F32, tag="warm_exp", name="warm_exp")
            nc.scalar.activation(warm_exp, warm_src[:, 0:1],
                                 mybir.ActivationFunctionType.Exp, scale=SCALE)
            warm_ps = ps_aux.tile([1, 256], F32, tag="aux", name="warm_ps",
                                  padded_shape=[128, 512])
            for _w in range(N_WARM):
                nc.tensor.matmul(warm_ps, warm_src[:, 0:1], warm_src,
                                 start=True, stop=True)
            for _w in range(8):
                nc.tensor.matmul(warm_ps[:, 0:1], warm_src[:, 0:1],
                                 warm_src[:, 0:1], start=True, stop=True)

            # ---- deferred-PE-work fill queue ------------------------------
            # (cost_ns, closure) FIFO; attention units pop ~STEP_BUDGET ns of
            # work between score matmuls.  Emission order == pop order, so
            # enqueue order must respect data deps.
            fill_q = []
            fq_stat = {"pushed": 0, "popped": 0}

            def fq_push(cost, fn):
                fill_q.append((cost, fn))
                fq_stat["pushed"] += 1

            def _fq_pop1():
                cost, fn = fill_q.pop(0)
                fq_stat["popped"] += 1
                fn()
                return cost

            def fq_pop(budget):
                spent = 0.0
                while fill_q and spent < budget:
                    spent += _fq_pop1()

            def fq_drain():
                while fill_q:
                    _fq_pop1()

            def fq_barrier(mark):
                """Ensure the first `mark` pushed items have been emitted
                (FIFO partial barrier for deadline-ordered work)."""
                while fq_stat["popped"] < mark and fill_q:
                    _fq_pop1()

            # ---- projection chains (as fine-grained queue items) ---------
            def chain(lhs_fn, rhs_fn, n_k, width, evict_fn, name):
                """Accumulating matmul chain split into 2-matmul queue items
                + an eviction item.  lhs_fn/rhs_fn map kt -> AP."""
                box = {}

                def piece(k0, k1):
                    def go():
                        if k0 == 0:
                            box["ps"] = ps_aux.tile(
                                [128, width], F32, tag="aux", name=name,
                                padded_shape=[128, 512])
                        ps = box["ps"]
                        for kt in range(k0, k1):
                            nc.tensor.matmul(
                                ps, lhs_fn(kt), rhs_fn(kt),
                                start=(kt == 0), stop=(kt == n_k - 1),
                            )
                    return go

                for k0 in range(0, n_k, 2):
                    k1 = min(k0 + 2, n_k)
                    fq_push((k1 - k0) * width * 0.42, piece(k0, k1))
                fq_push(0, lambda: evict_fn(box["ps"]))

            def qk_group(mt, chs=(0, 1), evict_on_act=False):
                """qkT[mt, ch-chunk] = (w_qk col-block mt)^T @ xT.
                Col blocks (pair-packed): mt=2p -> q of pair p, 2p+1 -> k.
                evict_on_act alternates the two chunks across ACT/DVE so the
                evictions (which gate the first scores) run in parallel."""
                for ch in chs:
                    def evict(ps, mt=mt, ch=ch):
                        dst = qkT[:, mt, ch * 512:(ch + 1) * 512]
                        if evict_on_act and ch == 0:
                            nc.scalar.copy(dst, ps)
                        else:
                            nc.vector.tensor_copy(dst, ps)
                    chain(
                        lambda kt, mt=mt: wqk[:, kt, mt * 128:(mt + 1) * 128],
                        lambda kt, ch=ch: xT[:, kt, ch * 512:(ch + 1) * 512],
                        6, 512, evict, f"qk_{mt}_{ch}",
                    )

            def v_group(i):
                """v rows-block i = x[i-block] @ w_v, strided into v_sb."""
                def evict(ps, i=i):
                    dst = v_sb[:, i, :].rearrange("p (h c) -> p h c", h=6)[:, :, 0:DH]
                    nc.vector.tensor_copy(dst, ps.rearrange("p (h c) -> p h c", h=6))
                chain(
                    lambda kt, i=i: xT[:, kt, i * 128:(i + 1) * 128],
                    lambda kt: wv[:, kt, :],
                    6, 384, evict, f"v_{i}",
                )

            # ---- attention ------------------------------------------------
            def emit_normalize_qt(p, ch, acc, qt, qi):
                """Single-qt normalize: groups (qi*2, qi*2+1) of `acc`."""
                dinv = stats.tile([128, 2], F32, tag="dinv",
                                  name=f"dinvq_{p}_{ch}_{qt}")
                dview = acc.rearrange("p (g c) -> p g c", g=4)[:, 2 * qi:2 * qi + 2, 64]
                with nc.allow_low_precision(
                    reason="softmax denominators are O(100); rounding of "
                    "1/denom is below the bf16 noise floor of the weave"
                ):
                    nc.vector.reciprocal(dinv, dview)
                stage = attn_pool.tile(
                    [128, 128], BF16, tag="attn", name=f"attnq_{p}_{ch}_{qt}"
                )
                for hp in range(2):
                    nc.vector.tensor_scalar_mul(
                        stage[:, hp * 64:hp * 64 + 64],
                        acc[:, (2 * qi + hp) * 128:(2 * qi + hp) * 128 + 64],
                        dinv[:, hp:hp + 1],
                    )
                return stage

            def emit_normalize(p, ch, acc, qts):
                """acc holds 4 groups (qt, hp) at 128-col stride, col 64 of
                each group = softmax denominator.  DVE: one strided
                reciprocal + 4 per-partition-scalar muls into bf16 staging."""
                dinv = stats.tile([128, 4], F32, tag="dinv",
                                  name=f"dinv_{p}_{ch}_{qts[0]}")
                dview = acc.rearrange("p (g c) -> p g c", g=4)[:, :, 64]
                with nc.allow_low_precision(
                    reason="softmax denominators are O(100); rounding of "
                    "1/denom is below the bf16 noise floor of the weave"
                ):
                    nc.vector.reciprocal(dinv, dview)
                stages = []
                for qi, qt in enumerate(qts):
                    stage = attn_pool.tile(
                        [128, 128], BF16, tag="attn", name=f"attn_{p}_{ch}_{qt}"
                    )
                    for hp in range(2):
                        g = qi * 2 + hp
                        nc.vector.tensor_scalar_mul(
                            stage[:, hp * 64:hp * 64 + 64],
                            acc[:, g * 128:g * 128 + 64],
                            dinv[:, g:g + 1],
                        )
                    stages.append((qt, stage))
                return stages

            def emit_transpose(p, ch, qt, stage, evict_act=False):
                """PE-transpose one [128q, 128d(2 heads)] staging tile into
                d-major attT columns; bf16 PSUM via bitcast.  Eviction on DVE
                (weave) or ACT (tail, where ACT is idle and DVE is the
                serial bottleneck)."""
                tp_f32 = ps_aux.tile([128, 64], F32, tag="aux",
                                     name=f"tp_{p}_{ch}_{qt}",
                                     padded_shape=[128, 512])
                tp = tp_f32.bitcast(BF16)
                nc.tensor.matmul(tp, stage, ident, is_transpose=True,
                                 start=True, stop=True)
                dst = attT[:, p, ch * 512 + qt * 128:ch * 512 + (qt + 1) * 128]
                if evict_act:
                    nc.scalar.copy(dst, tp)
                else:
                    nc.vector.tensor_copy(dst, tp)

            def enqueue_av(prev, post_x=(), post_y=()):
                """Enqueue the AV burst + normalize + transposes for a
                finished unit.  PSUM groups zero a whole bank on start, so
                the 4 (qt,hp) groups of each bank run sequentially (each
                fully accumulated over i=0..7 before the next starts).
                `post_x`/`post_y` callbacks push follow-up work right behind
                each bank's normalize+transposes (used by the tail so each
                output row's final fires as soon as its attT columns land)."""
                pp, pch, ppts = prev
                boxes = {}

                def alloc(bank):
                    def go():
                        boxes[bank] = ps_acc.tile(
                            [128, 512], F32, tag="acc", name=f"acc{bank}_{pp}_{pch}"
                        )
                    return go

                def av_group(bank, g, qt, hp):
                    def go():
                        acc = boxes[bank]
                        h = 2 * pp + hp
                        for i in range(8):
                            nc.tensor.matmul(
                                acc[:, g * 128:g * 128 + 65],
                                ppts[i][:, hp * 512 + qt * 128:hp * 512 + (qt + 1) * 128],
                                v_sb[:, i, h * 65:h * 65 + 65],
                                start=(i == 0),
                                stop=(i == 7),
                            )
                    return go

                stage_box = {}

                def norm(bank, qts):
                    def go():
                        stage_box[bank] = emit_normalize(pp, pch, boxes[bank], qts)
                    return go

                def tp(bank):
                    def go():
                        for qt, stage in stage_box[bank]:
                            emit_transpose(pp, pch, qt, stage)
                    return go

                # The normalize item carries an inflated cost so the pop
                # loop breaks between it and the transposes — the next score
                # matmul then fills the PE pipeline while the DVE muls run
                # (the transposes read their output, so emitting them
                # back-to-back would head-of-line block PE on DVE latency).
                fq_push(0, alloc("X"))
                for g, (qt, hp) in enumerate(((0, 0), (0, 1), (1, 0), (1, 1))):
                    fq_push(8 * 65 * 0.42, av_group("X", g, qt, hp))
                fq_push(STEP_BUDGET, norm("X", (0, 1)))
                fq_push(110, tp("X"))
                for fn in post_x:
                    fn()
                fq_push(0, alloc("Y"))
                for g, (qt, hp) in enumerate(((2, 0), (2, 1), (3, 0), (3, 1))):
                    fq_push(8 * 65 * 0.42, av_group("Y", g, qt, hp))
                fq_push(STEP_BUDGET, norm("Y", (2, 3)))
                fq_push(110, tp("Y"))
                for fn in post_y:
                    fn()

            def enqueue_av_tail(prev, finals, pre=()):
                """Tail variant of enqueue_av: per-qt stagger so each output
                row's final (j=2 + add + DMA) fires as soon as that qt's two
                AV groups, normalize, and transpose land — instead of
                waiting for a whole bank of four."""
                pp, pch, ppts = prev
                boxes = {}

                def alloc(bank):
                    def go():
                        boxes[bank] = ps_acc.tile(
                            [128, 512], F32, tag="acc", name=f"acct{bank}_{pp}_{pch}"
                        )
                    return go

                def av_group(bank, g, qt, hp):
                    def go():
                        acc = boxes[bank]
                        h = 2 * pp + hp
                        for i in range(8):
                            nc.tensor.matmul(
                                acc[:, g * 128:g * 128 + 65],
                                ppts[i][:, hp * 512 + qt * 128:hp * 512 + (qt + 1) * 128],
                                v_sb[:, i, h * 65:h * 65 + 65],
                                start=(i == 0),
                                stop=(i == 7),
                            )
                    return go

                stage_box = {}

                def norm_qt(bank, qt, qi):
                    def go():
                        stage_box[qt] = emit_normalize_qt(pp, pch, boxes[bank], qt, qi)
                    return go

                def tp_qt(qt):
                    def go():
                        emit_transpose(pp, pch, qt, stage_box[qt],
                                       evict_act=(qt % 2 == 0))
                    return go

                # All AV bursts + normalizes first (PE work with no
                # cross-engine stalls; the DVE normalizes overlap), then the
                # per-row transpose -> final -> DMA chains, evictions
                # alternating ACT/DVE so neither engine serializes the tail.
                # Bank X holds qt0/qt1's groups, bank Y qt2/qt3's; a bank's
                # second qt starts only after its first stopped (sequential
                # accumulation groups), and the normalize reads survive the
                # later start because PSUM zeroing is lazy.
                for qt in range(4):
                    bank = "X" if qt < 2 else "Y"
                    qi = qt % 2
                    if qi == 0:
                        fq_push(0, alloc(bank))
                    for hp in range(2):
                        fq_push(8 * 65 * 0.42, av_group(bank, 2 * qi + hp, qt, hp))
                    fq_push(0, norm_qt(bank, qt, qi))
                    if qt == 0:
                        # independent leftovers slot in behind the first AV
                        # burst, covering the exp-drain window
                        for fn in pre:
                            fn()
                for qt in range(4):
                    fq_push(60, tp_qt(qt))
                    fq_push(0, finals[qt])

            def attention_unit(p, ch, prev, extra=(), budget=STEP_BUDGET):
                """One (pair, query-chunk) unit: 8 x (scores both heads into
                a 2-bank PSUM tile, one 1024-wide exp).  The previous unit's
                AV/normalize/transposes are enqueued first so they fill this
                unit's PE bubbles (the scores pipeline is ACT-bound).
                `extra` closures are enqueued after the AV items — for work
                that depends on the previous unit's attT transposes."""
                qt_ = 2 * p       # qkT tile of this pair's q
                kt_ = 2 * p + 1   # qkT tile of this pair's k
                pts = []

                def score_exp(i):
                    s = ps_s.tile([128, 1024], F32, tag="s", name=f"s_{p}_{ch}_{i}")
                    for hp in range(2):
                        lo, hi = hp * 64, hp * 64 + 64
                        nc.tensor.matmul(
                            s[:, hp * 512:(hp + 1) * 512],
                            qkT[lo:hi, kt_, i * 128:(i + 1) * 128],
                            qkT[lo:hi, qt_, ch * 512:(ch + 1) * 512],
                            start=True,
                            stop=True,
                        )
                    pt = pt_pool.tile([128, 1024], BF16, tag="pt",
                                      name=f"pt_{p}_{ch}_{i}")
                    nc.scalar.activation(pt, s, mybir.ActivationFunctionType.Exp,
                                         scale=SCALE)
                    pts.append(pt)

                score_exp(0)
                score_exp(1)
                if prev is not None:
                    enqueue_av(prev)
                for fn in extra:
                    fn()
                for i in range(2, 8):
                    score_exp(i)
                    fq_pop(STEP_BUDGET)
                fq_pop(2 * STEP_BUDGET)
                return (p, ch, pts)

            # ---- out-projection ------------------------------------------
            CHUNKS = ((0, 512), (512, 256))
            _osb_cache = {}

            def out_group(i, c):
                """Chunk c of row-block i (rows 0..3 path): 3 j-matmuls +
                DVE evict; c==1 flushes the row's 768-wide DMA."""
                c0, cw = CHUNKS[c]
                if c == 0:
                    osb = outsb.tile([128, 768], F32, tag="osb", name=f"osb_{i}")
                    _osb_cache[i] = osb
                else:
                    osb = _osb_cache.pop(i)

                def evict(ps, i=i, c=c, osb=osb):
                    nc.vector.tensor_copy(osb[:, c0:c0 + cw], ps)
                    if c == 1:
                        nc.sync.dma_start(
                            out=out_d[i * 128:(i + 1) * 128, :], in_=osb)
                chain(
                    lambda j, i=i: attT[:, j, i * 128:(i + 1) * 128],
                    lambda j, c0=c0, cw=cw: wo[:, j, c0:c0 + cw],
                    3, cw, evict, f"o_{i}_{c}",
                )

            def out_partial_chain(i, c):
                """j=0,1 of (row-block i, chunk c) into out_partial; c==1
                flushes the row's partial straight to DRAM (the host adds
                the j=2 remainder from out2)."""
                c0, cw = CHUNKS[c]

                def evict(ps, i=i, c=c):
                    nc.vector.tensor_copy(out_partial[:, i - 4, c0:c0 + cw], ps)
                    if c == 1:
                        nc.sync.dma_start(
                            out=out_d[i * 128:(i + 1) * 128, :],
                            in_=out_partial[:, i - 4, :],
                        )
                chain(
                    lambda j, i=i: attT[:, j, i * 128:(i + 1) * 128],
                    lambda j, c0=c0, cw=cw: wo[:, j, c0:c0 + cw],
                    2, cw, evict, f"pp_{i}_{c}",
                )

            def out_final(i):
                """j=2-only remainder of row-block i -> out2 (host adds it
                onto the j=0,1 partials).  Evictions alternate ACT/DVE by
                row so neither engine serializes the tail.  Emits directly
                (tail drain context)."""
                osb = outsb.tile([128, 768], F32, tag="osb", name=f"osbf_{i}")
                r0 = (i - 4) * 128
                for c, (c0, cw) in enumerate(CHUNKS):
                    ps = ps_aux.tile([128, cw], F32, tag="aux", name=f"f_{i}_{c}",
                                     padded_shape=[128, 512])
                    nc.tensor.matmul(
                        ps,
                        attT[:, 2, i * 128:(i + 1) * 128],
                        wo[:, 2, c0:c0 + cw],
                        start=True,
                        stop=True,
                    )
                    if i % 2 == 0:
                        nc.scalar.copy(osb[:, c0:c0 + cw], ps)
                    else:
                        nc.vector.tensor_copy(osb[:, c0:c0 + cw], ps)
                    if i == 7:
                        # flush per chunk so the kernel-end drain only waits
                        # on the short final transfer
                        nc.sync.dma_start(
                            out=out2_d[r0:r0 + 128, c0:c0 + cw],
                            in_=osb[:, c0:c0 + cw],
                        )
                if i != 7:
                    nc.sync.dma_start(out=out2_d[r0:r0 + 128, :], in_=osb)

            # ---- schedule -------------------------------------------------
            # Pre-weave: the minimum projections unit (0,0) needs — q pair 0
            # chunk 0 and k pair 0 both chunks (scores sweep all key blocks).
            # Alternating eviction engines so neither DVE nor ACT gates the
            # first scores.  q chunk 1 flows through the queue.
            qk_group(0, chs=(0,))
            qk_group(1, evict_on_act=True)
            fq_drain()

            # Weave. Query-chunk-0 units first: once (0,0),(1,0),(2,0) are
            # done, output row-blocks 0..3 are fully determined, so their
            # out-projection (and DMA) overlaps the chunk-1 units.  v, the
            # remaining qk projections, AV bursts, and out chains all flow
            # through the fill queue; a full drain before each unit bounds
            # the backlog and guarantees every unit's inputs (its pair's qkT
            # chains, v for the AV bursts) are emitted before its scores.
            # Queue order is deadline order: pair-1 then pair-2 projections
            # (they gate units (1,0)/(2,0)'s scores — the old full drains
            # emitted them at boundaries BEHIND the v chains, opening 3-9us
            # ACT holes), then q pair-0 chunk 1, then v (only consumed by
            # the first AV burst mid-unit-(1,0), which FIFO-orders after it).
            qk_group(2)
            qk_group(3)
            after_qk23 = fq_stat["pushed"]
            qk_group(4)
            qk_group(5)
            after_qk45 = fq_stat["pushed"]
            qk_group(0, chs=(1,))
            for i in range(8):
                v_group(i)
            prev = attention_unit(0, 0, None)
            fq_barrier(after_qk23)
            prev = attention_unit(1, 0, prev)
            fq_barrier(after_qk45)
            prev = attention_unit(2, 0, prev)
            # out row-blocks 0..3 depend only on the ch-0 attT columns; the
            # last of those (pair 2) is transposed inside unit (0,1) by its
            # AV items, so the row chains can fill units (0,1) and (1,1) —
            # without them the late units' fill queues run dry and PE both
            # idles and drops to the cold p-state.
            prev = attention_unit(0, 1, prev, extra=[
                lambda i=i, c=c: out_group(i, c)
                for i in (0, 1) for c in (0, 1)
            ])
            prev = attention_unit(1, 1, prev, extra=[
                lambda i=i, c=c: out_group(i, c)
                for i in (2, 3) for c in (0, 1)
            ])
            # rows 4..7 j=0,1 partials read attT pair-0/1 ch-1; pair 1 is
            # only transposed inside unit (2,1) — enqueue them behind its AV
            # items.  The j=2 finals then ride the tail AV's post hooks so
            # each row's add+DMA fires as soon as its attT columns land.
            prev = attention_unit(2, 1, prev, extra=[
                lambda i=i, c=c: out_partial_chain(i, c)
                for i in (4, 5) for c in (0, 1)
            ])
            out_partial_chain(6, 0)
            out_partial_chain(6, 1)
            enqueue_av_tail(prev, [lambda i=i: out_final(i) for i in (4, 5, 6, 7)],
                            pre=[lambda: out_partial_chain(7, 0),
                                 lambda: out_partial_chain(7, 1)])
            fq_drain()

    nc.compile()
    return nc


_NC_CACHE = {}


def _get_nc():
    if "nc" not in _NC_CACHE:
        _NC_CACHE["nc"] = build_nc()
    return _NC_CACHE["nc"]


def kernel(x, w_qkv, w_out, b_out):
    import ml_dtypes

    def bf16(a):
        return np.ascontiguousarray(
            np.asarray(a, dtype=np.float32).astype(ml_dtypes.bfloat16)
        )

    x = np.asarray(x, dtype=np.float32)
    w_qkv = np.asarray(w_qkv, dtype=np.float32)
    w_out = np.asarray(w_out, dtype=np.float32)
    b_out = np.asarray(b_out, dtype=np.float32)

    nc = _get_nc()
    ones_col = np.ones((128, 64), dtype=np.float32)
    ident = np.eye(128, dtype=np.float32)
    in_maps = []
    for c in range(NCORES):
        b = c // 2
        hs = (c % 2) * HEADS_PER_CORE
        q_cols = w_qkv[:, hs * DH:(hs + 6) * DH]
        k_cols = w_qkv[:, 768 + hs * DH:768 + (hs + 6) * DH]
        # pair-packed: [q_p0 | k_p0 | q_p1 | k_p1 | q_p2 | k_p2], 128 each
        wqk_packed = np.concatenate(
            [blk for p in range(3)
             for blk in (q_cols[:, p * 128:(p + 1) * 128],
                         k_cols[:, p * 128:(p + 1) * 128])],
            axis=1,
        )
        in_maps.append({
            "xT": bf16(x[b].T),
            "w_qk": bf16(wqk_packed),
            "w_v": bf16(w_qkv[:, 1536 + hs * DH:1536 + (hs + 6) * DH]),
            "w_o": bf16(w_out[hs * DH:(hs + 6) * DH, :]),
            "ones_col": bf16(ones_col),
            "ident": bf16(ident),
        })

    res = run_bass_kernel_spmd(nc, in_maps, core_ids=list(range(NCORES))).results

    out = np.empty((4, N, DIM), dtype=np.float32)
    for b in range(4):
        out[b] = res[2 * b]["out"] + res[2 * b + 1]["out"] + b_out
        # rows 512.. left the device as (j01 partials, j2 remainder)
        out[b, N // 2:] += res[2 * b]["out2"] + res[2 * b + 1]["out2"]
    return out


# revision 65
# speedup vs baseline: 1.0767x; 1.0081x over previous
"""Multi-head attention forward (B=4, N=1024, D=768, H=12, dh=64) on 8 TRN2 cores.

Sharding: (batch, head-group) — core c handles batch b = c//2 and heads
hs..hs+5 where hs = (c%2)*6.  Each core computes its 6 heads' contribution
to out[b] = attn(x[b]) @ W_out_rows(for its heads); host sums the two
partials per batch and adds the bias (the "all-reduce after final linear").

All SBUF tensors are bf16 (halves DMA + keeps matmuls at 1 cycle/row for
any moving-dim size); PSUM accumulation stays fp32.

Per-core dataflow:
  qkT  [768,1024] = w_qk^T @ x^T          (d-major q,k — feeds scores;
                                           w_qk cols pair-packed
                                           [q_p0|k_p0|q_p1|k_p1|q_p2|k_p2])
  v    [1024,390] = x @ w_v (+ ones col)  (n-major v — feeds AV)
  S^T  [128k,2*512q]/(pair,i) = k_h @ q_h^T  (keys on partitions; the two
                                           heads' q-chunks side by side in
                                           one 2-bank PSUM tile)
  P^T  = exp(S^T * scale)                 (ONE [128,1024] ACT op per (pair,i):
                                           no max-sub — scores ~ N(0,1))
  o    [128q,65]/(head,qtile) = P_slice^T^T @ [v_h|1]  (q-major AV: full 128
                                           PE rows; col 64 = denominator)
  attn = o[:,0:64] * (1/o[:,64])          (per-partition scalar broadcast on
                                           DVE — no broadcast matmul needed)
  attT = PE-transpose(attn)               (back to d-major for the out proj)
  out  [1024,768] = attT^T @ w_o          (partial; host all-reduce)

PSUM accumulation groups zero a whole 2KB bank on start, so the 8 q-major
AV accumulators of a unit (4 per bank x 2 banks) must run sequentially
within each bank: the unit's AV is a 64-matmul burst executed in the NEXT
unit's window (PE is idle there waiting on the exp pipeline).  All deferred
PE work — AV bursts, transposes, projection / out-projection chain pieces —
flows through a FIFO fill queue popped between score matmuls, keeping the
tensor engine saturated at ~2-matmul granularity so the ACT exp cadence
never starves.
"""
import os
import sys

sys.path.insert(0, "/opt/trn_rl_repo")

# The kernel needs the axon-tunneled TRN2 PJRT backend; a JAX_PLATFORMS=cpu
# pin (common for reference-side jax) would hide the NeuronCores.
if os.environ.get("JAX_PLATFORMS", "").strip() == "cpu":
    del os.environ["JAX_PLATFORMS"]

import numpy as np
import concourse.bass as bass
import concourse.bacc as bacc
import concourse.tile as tile
from concourse import mybir
from concourse.bass_utils import run_bass_kernel_spmd
from contextlib import ExitStack

F32 = mybir.dt.float32
BF16 = mybir.dt.bfloat16

DIM = 768
N = 1024
HEADS_PER_CORE = 6
DH = 64
SCALE = DH ** -0.5
NCORES = 8
N_WARM = int(os.environ.get("ATTN_N_WARM", "32"))
STEP_BUDGET = float(os.environ.get("ATTN_STEP_BUDGET", "640"))
LATE_BUDGET = float(os.environ.get("ATTN_LATE_BUDGET", "1000"))


def build_nc():
    DT = BF16
    nc = bacc.Bacc("TRN2", target_bir_lowering=False, debug=False)

    xT_d = nc.declare_dram_parameter("xT", [DIM, N], DT, isOutput=False)
    wqk_d = nc.declare_dram_parameter("w_qk", [DIM, 768], DT, isOutput=False)
    wv_d = nc.declare_dram_parameter("w_v", [DIM, 384], DT, isOutput=False)
    wo_d = nc.declare_dram_parameter("w_o", [384, DIM], DT, isOutput=False)
    ones_d = nc.declare_dram_parameter("ones_col", [128, 64], DT, isOutput=False)
    ident_d = nc.declare_dram_parameter("ident", [128, 128], DT, isOutput=False)
    out_d = nc.declare_dram_parameter("out", [N, DIM], F32, isOutput=True)
    # rows 4..7 leave the device in two pieces summed on the host: j=0,1
    # partials (DMA'd during the last unit) go to out, the j=2 remainder
    # (tail) to out2 — killing the on-device combining adds, which were the
    # serial-DVE bottleneck of the tail.
    out2_d = nc.declare_dram_parameter("out2", [N // 2, DIM], F32, isOutput=True)

    with tile.TileContext(nc) as tc:
        with ExitStack() as ctx:
            persist = ctx.enter_context(tc.tile_pool(name="persist", bufs=1))
            # pt ring sized for ~3 units in flight so a unit's AV burst may
            # spill two units ahead without blocking the exp pipeline
            pt_pool = ctx.enter_context(tc.tile_pool(name="pt", bufs=26))
            attn_pool = ctx.enter_context(tc.tile_pool(name="attn", bufs=6))
            stats = ctx.enter_context(tc.tile_pool(name="stats", bufs=4))
            outsb = ctx.enter_context(tc.tile_pool(name="outsb", bufs=4))
            # PSUM slots are bank-granular: 2 x 2-bank score tiles + 2 x
            # 1-bank AV accumulators + 2 x 1-bank aux slots = all 8 banks.
            ps_s = ctx.enter_context(tc.tile_pool(name="ps_s", bufs=2, space="PSUM"))
            ps_acc = ctx.enter_context(tc.tile_pool(name="ps_acc", bufs=2, space="PSUM"))
            ps_aux = ctx.enter_context(tc.tile_pool(name="ps_aux", bufs=2, space="PSUM"))

            xT = persist.tile([128, 6, N], DT)
            wqk = persist.tile([128, 6, 768], DT)
            wv = persist.tile([128, 6, 384], DT)
            wo = persist.tile([128, 3, 768], DT)
            qkT = persist.tile([128, 6, N], DT)
            v_sb = persist.tile([128, 8, 6 * 65], DT)
            attT = persist.tile([128, 3, N], DT)
            ident = persist.tile([128, 128], DT)
            out_partial = persist.tile([128, 4, DIM], F32)
            warm_src = persist.tile([128, 256], DT)

            # Input DMAs, one batched transfer per tensor (each dma_start
            # costs ~0.6us on the shared HWDGE generator + ~0.9us sem
            # propagation, so fewer/bigger is strictly better).  Transfer
            # order on the exclusive DMA device: wqk pair-0 cols + xT first
            # (they gate the first projection chains), then w_v, the rest of
            # w_qk, w_o.  Two queues (SP + ACT) halve issue latency.
            # All input DMAs on the SP queue in strict priority order — the
            # DMA device is exclusive, so a second queue would only let a
            # low-priority transfer cut ahead of the chain-gating wqk/xT
            # pair (issue costs pipeline ahead of the transfers anyway).
            nc.sync.dma_start(
                out=wqk[:, :, 0:256],
                in_=wqk_d[:, 0:256].rearrange("(k p) n -> p k n", p=128),
            )
            nc.sync.dma_start(
                out=xT, in_=xT_d.rearrange("(k p) n -> p k n", p=128)
            )
            nc.sync.dma_start(
                out=wv, in_=wv_d.rearrange("(k p) n -> p k n", p=128)
            )
            nc.sync.dma_start(
                out=wqk[:, :, 256:768],
                in_=wqk_d[:, 256:768].rearrange("(k p) n -> p k n", p=128),
            )
            nc.sync.dma_start(
                out=wo, in_=wo_d.rearrange("(k p) n -> p k n", p=128)
            )
            ones_stage = persist.tile([128, 64], DT)
            nc.sync.dma_start(out=ones_stage, in_=ones_d[:, :])
            nc.sync.dma_start(out=ident, in_=ident_d[:, :])
            # ones: v_sb[:, i, h*65 + 64] = 1.0 for all (i, h). The strided
            # scatter runs on the otherwise-idle GPSIMD (SBUF->SBUF is its
            # one legal niche) so it can never head-of-line block DVE's
            # projection evictions; as a DMA it would cost 6144 min-size
            # descriptors.
            v_ones_view = v_sb.rearrange("p i (h c) -> p i h c", h=6)[:, :, :, 64]
            nc.gpsimd.tensor_copy(
                v_ones_view, ones_stage[:, 0:48].rearrange("p (i h) -> p i h", i=8)
            )

            # PE clock warm-up: the tensor engine idles during the input DMA
            # window and would otherwise start the projection at the cold
            # p-state (and the ramp needs 3us of continuous execution to
            # reach full clock).  Matmuls against a GPSIMD-memset SBUF tile
            # keep PE busy across the DMA window with no data dependencies;
            # the trailing N=1 ones keep the tail cheap so the first real
            # chain isn't delayed.
            nc.gpsimd.memset(warm_src, 1.0)
            # Preload the ACT exp table during the DMA window (first real exp
            # would otherwise pay the ~1.3us table load at weave start).
            warm_exp = stats.tile([128, 1], F32, tag="warm_exp", name="warm_exp")
            nc.scalar.activation(warm_exp, warm_src[:, 0:1],
                                 mybir.ActivationFunctionType.Exp, scale=SCALE)
            warm_ps = ps_aux.tile([1, 256], F32, tag="aux", name="warm_ps",
                                  padded_shape=[128, 512])
            for _w in range(N_WARM):
                nc.tensor.matmul(warm_ps, warm_src[:, 0:1], warm_src,
                                 start=True, stop=True)
            for _w in range(8):
                nc.tensor.matmul(warm_ps[:, 0:1], warm_src[:, 0:1],
                                 warm_src[:, 0:1], start=True, stop=True)

            # ---- deferred-PE-work fill queue ------------------------------
            # (cost_ns, closure) FIFO; attention units pop ~STEP_BUDGET ns of
            # work between score matmuls.  Emission order == pop order, so
            # enqueue order must respect data deps.
            fill_q = []
            fq_stat = {"pushed": 0, "popped": 0}

            def fq_push(cost, fn):
                fill_q.append((cost, fn))
                fq_stat["pushed"] += 1

            def _fq_pop1():
                cost, fn = fill_q.pop(0)
                fq_stat["popped"] += 1
                fn()
                return cost

            def fq_pop(budget):
                spent = 0.0
                while fill_q and spent < budget:
                    spent += _fq_pop1()

            def fq_drain():
                while fill_q:
                    _fq_pop1()

            def fq_barrier(mark):
                """Ensure the first `mark` pushed items have been emitted
                (FIFO partial barrier for deadline-ordered work)."""
                while fq_stat["popped"] < mark and fill_q:
                    _fq_pop1()

            # ---- projection chains (as fine-grained queue items) ---------
            def chain(lhs_fn, rhs_fn, n_k, width, evict_fn, name):
                """Accumulating matmul chain split into 2-matmul queue items
                + an eviction item.  lhs_fn/rhs_fn map kt -> AP."""
                box = {}

                def piece(k0, k1):
                    def go():
                        if k0 == 0:
                            box["ps"] = ps_aux.tile(
                                [128, width], F32, tag="aux", name=name,
                                padded_shape=[128, 512])
                        ps = box["ps"]
                        for kt in range(k0, k1):
                            nc.tensor.matmul(
                                ps, lhs_fn(kt), rhs_fn(kt),
                                start=(kt == 0), stop=(kt == n_k - 1),
                            )
                    return go

                for k0 in range(0, n_k, 2):
                    k1 = min(k0 + 2, n_k)
                    fq_push((k1 - k0) * width * 0.42, piece(k0, k1))
                fq_push(0, lambda: evict_fn(box["ps"]))

            def qk_group(mt, chs=(0, 1), evict_on_act=False):
                """qkT[mt, ch-chunk] = (w_qk col-block mt)^T @ xT.
                Col blocks (pair-packed): mt=2p -> q of pair p, 2p+1 -> k.
                evict_on_act alternates the two chunks across ACT/DVE so the
                evictions (which gate the first scores) run in parallel."""
                for ch in chs:
                    def evict(ps, mt=mt, ch=ch):
                        dst = qkT[:, mt, ch * 512:(ch + 1) * 512]
                        if evict_on_act and ch == 0:
                            nc.scalar.copy(dst, ps)
                        else:
                            nc.vector.tensor_copy(dst, ps)
                    chain(
                        lambda kt, mt=mt: wqk[:, kt, mt * 128:(mt + 1) * 128],
                        lambda kt, ch=ch: xT[:, kt, ch * 512:(ch + 1) * 512],
                        6, 512, evict, f"qk_{mt}_{ch}",
                    )

            def v_group(i):
                """v rows-block i = x[i-block] @ w_v, strided into v_sb."""
                def evict(ps, i=i):
                    dst = v_sb[:, i, :].rearrange("p (h c) -> p h c", h=6)[:, :, 0:DH]
                    nc.vector.tensor_copy(dst, ps.rearrange("p (h c) -> p h c", h=6))
                chain(
                    lambda kt, i=i: xT[:, kt, i * 128:(i + 1) * 128],
                    lambda kt: wv[:, kt, :],
                    6, 384, evict, f"v_{i}",
                )

            # ---- attention ------------------------------------------------
            def emit_normalize_qt(p, ch, acc, qt, qi):
                """Single-qt normalize: groups (qi*2, qi*2+1) of `acc`."""
                dinv = stats.tile([128, 2], F32, tag="dinv",
                                  name=f"dinvq_{p}_{ch}_{qt}")
                dview = acc.rearrange("p (g c) -> p g c", g=4)[:, 2 * qi:2 * qi + 2, 64]
                with nc.allow_low_precision(
                    reason="softmax denominators are O(100); rounding of "
                    "1/denom is below the bf16 noise floor of the weave"
                ):
                    nc.vector.reciprocal(dinv, dview)
                stage = attn_pool.tile(
                    [128, 128], BF16, tag="attn", name=f"attnq_{p}_{ch}_{qt}"
                )
                for hp in range(2):
                    nc.vector.tensor_scalar_mul(
                        stage[:, hp * 64:hp * 64 + 64],
                        acc[:, (2 * qi + hp) * 128:(2 * qi + hp) * 128 + 64],
                        dinv[:, hp:hp + 1],
                    )
                return stage

            def emit_normalize(p, ch, acc, qts):
                """acc holds 4 groups (qt, hp) at 128-col stride, col 64 of
                each group = softmax denominator.  DVE: one strided
                reciprocal + 4 per-partition-scalar muls into bf16 staging."""
                dinv = stats.tile([128, 4], F32, tag="dinv",
                                  name=f"dinv_{p}_{ch}_{qts[0]}")
                dview = acc.rearrange("p (g c) -> p g c", g=4)[:, :, 64]
                with nc.allow_low_precision(
                    reason="softmax denominators are O(100); rounding of "
                    "1/denom is below the bf16 noise floor of the weave"
                ):
                    nc.vector.reciprocal(dinv, dview)
                stages = []
                for qi, qt in enumerate(qts):
                    stage = attn_pool.tile(
                        [128, 128], BF16, tag="attn", name=f"attn_{p}_{ch}_{qt}"
                    )
                    for hp in range(2):
                        g = qi * 2 + hp
                        nc.vector.tensor_scalar_mul(
                            stage[:, hp * 64:hp * 64 + 64],
                            acc[:, g * 128:g * 128 + 64],
                            dinv[:, g:g + 1],
                        )
                    stages.append((qt, stage))
                return stages

            def emit_transpose(p, ch, qt, stage, evict_act=False):
                """PE-transpose one [128q, 128d(2 heads)] staging tile into
                d-major attT columns; bf16 PSUM via bitcast.  Eviction on DVE
                (weave) or ACT (tail, where ACT is idle and DVE is the
                serial bottleneck)."""
                tp_f32 = ps_aux.tile([128, 64], F32, tag="aux",
                                     name=f"tp_{p}_{ch}_{qt}",
                                     padded_shape=[128, 512])
                tp = tp_f32.bitcast(BF16)
                nc.tensor.matmul(tp, stage, ident, is_transpose=True,
                                 start=True, stop=True)
                dst = attT[:, p, ch * 512 + qt * 128:ch * 512 + (qt + 1) * 128]
                if evict_act:
                    nc.scalar.copy(dst, tp)
                else:
                    nc.vector.tensor_copy(dst, tp)

            def enqueue_av(prev, post_x=(), post_y=()):
                """Enqueue the AV burst + normalize + transposes for a
                finished unit.  PSUM groups zero a whole bank on start, so
                the 4 (qt,hp) groups of each bank run sequentially (each
                fully accumulated over i=0..7 before the next starts).
                `post_x`/`post_y` callbacks push follow-up work right behind
                each bank's normalize+transposes (used by the tail so each
                output row's final fires as soon as its attT columns land)."""
                pp, pch, ppts = prev
                boxes = {}

                def alloc(bank):
                    def go():
                        boxes[bank] = ps_acc.tile(
                            [128, 512], F32, tag="acc", name=f"acc{bank}_{pp}_{pch}"
                        )
                    return go

                def av_group(bank, g, qt, hp):
                    def go():
                        acc = boxes[bank]
                        h = 2 * pp + hp
                        for i in range(8):
                            nc.tensor.matmul(
                                acc[:, g * 128:g * 128 + 65],
                                ppts[i][:, hp * 512 + qt * 128:hp * 512 + (qt + 1) * 128],
                                v_sb[:, i, h * 65:h * 65 + 65],
                                start=(i == 0),
                                stop=(i == 7),
                            )
                    return go

                stage_box = {}

                def norm(bank, qts):
                    def go():
                        stage_box[bank] = emit_normalize(pp, pch, boxes[bank], qts)
                    return go

                def tp(bank):
                    def go():
                        for qt, stage in stage_box[bank]:
                            emit_transpose(pp, pch, qt, stage)
                    return go

                # The normalize item carries an inflated cost so the pop
                # loop breaks between it and the transposes — the next score
                # matmul then fills the PE pipeline while the DVE muls run
                # (the transposes read their output, so emitting them
                # back-to-back would head-of-line block PE on DVE latency).
                fq_push(0, alloc("X"))
                for g, (qt, hp) in enumerate(((0, 0), (0, 1), (1, 0), (1, 1))):
                    fq_push(8 * 65 * 0.42, av_group("X", g, qt, hp))
                fq_push(STEP_BUDGET, norm("X", (0, 1)))
                fq_push(110, tp("X"))
                for fn in post_x:
                    fn()
                fq_push(0, alloc("Y"))
                for g, (qt, hp) in enumerate(((2, 0), (2, 1), (3, 0), (3, 1))):
                    fq_push(8 * 65 * 0.42, av_group("Y", g, qt, hp))
                fq_push(STEP_BUDGET, norm("Y", (2, 3)))
                fq_push(110, tp("Y"))
                for fn in post_y:
                    fn()

            def enqueue_av_tail(prev, finals, pre=()):
                """Tail variant of enqueue_av: per-qt stagger so each output
                row's final (j=2 + add + DMA) fires as soon as that qt's two
                AV groups, normalize, and transpose land — instead of
                waiting for a whole bank of four."""
                pp, pch, ppts = prev
                boxes = {}

                def alloc(bank):
                    def go():
                        boxes[bank] = ps_acc.tile(
                            [128, 512], F32, tag="acc", name=f"acct{bank}_{pp}_{pch}"
                        )
                    return go

                def av_group(bank, g, qt, hp):
                    def go():
                        acc = boxes[bank]
                        h = 2 * pp + hp
                        for i in range(8):
                            nc.tensor.matmul(
                                acc[:, g * 128:g * 128 + 65],
                                ppts[i][:, hp * 512 + qt * 128:hp * 512 + (qt + 1) * 128],
                                v_sb[:, i, h * 65:h * 65 + 65],
                                start=(i == 0),
                                stop=(i == 7),
                            )
                    return go

                stage_box = {}

                def norm_qt(bank, qt, qi):
                    def go():
                        stage_box[qt] = emit_normalize_qt(pp, pch, boxes[bank], qt, qi)
                    return go

                def tp_qt(qt):
                    def go():
                        emit_transpose(pp, pch, qt, stage_box[qt],
                                       evict_act=(qt % 2 == 0))
                    return go

                # All AV bursts + normalizes first (PE work with no
                # cross-engine stalls; the DVE normalizes overlap), then the
                # per-row transpose -> final -> DMA chains, evictions
                # alternating ACT/DVE so neither engine serializes the tail.
                # Bank X holds qt0/qt1's groups, bank Y qt2/qt3's; a bank's
                # second qt starts only after its first stopped (sequential
                # accumulation groups), and the normalize reads survive the
                # later start because PSUM zeroing is lazy.
                for qt in range(4):
                    bank = "X" if qt < 2 else "Y"
                    qi = qt % 2
                    if qi == 0:
                        fq_push(0, alloc(bank))
                    for hp in range(2):
                        fq_push(8 * 65 * 0.42, av_group(bank, 2 * qi + hp, qt, hp))
                    fq_push(0, norm_qt(bank, qt, qi))
                    if qt == 0:
                        # independent leftovers slot in behind the first AV
                        # burst, covering the exp-drain window
                        for fn in pre:
                            fn()
                for qt in range(4):
                    fq_push(60, tp_qt(qt))
                    fq_push(0, finals[qt])

            def attention_unit(p, ch, prev, extra=(), budget=STEP_BUDGET):
                """One (pair, query-chunk) unit: 8 x (scores both heads into
                a 2-bank PSUM tile, one 1024-wide exp).  The previous unit's
                AV/normalize/transposes are enqueued first so they fill this
                unit's PE bubbles (the scores pipeline is ACT-bound).
                `extra` closures are enqueued after the AV items — for work
                that depends on the previous unit's attT transposes."""
                qt_ = 2 * p       # qkT tile of this pair's q
                kt_ = 2 * p + 1   # qkT tile of this pair's k
                pts = []

                def score_exp(i):
                    s = ps_s.tile([128, 1024], F32, tag="s", name=f"s_{p}_{ch}_{i}")
                    for hp in range(2):
                        lo, hi = hp * 64, hp * 64 + 64
                        nc.tensor.matmul(
                            s[:, hp * 512:(hp + 1) * 512],
                            qkT[lo:hi, kt_, i * 128:(i + 1) * 128],
                            qkT[lo:hi, qt_, ch * 512:(ch + 1) * 512],
                            start=True,
                            stop=True,
                        )
                    pt = pt_pool.tile([128, 1024], BF16, tag="pt",
                                      name=f"pt_{p}_{ch}_{i}")
                    nc.scalar.activation(pt, s, mybir.ActivationFunctionType.Exp,
                                         scale=SCALE)
                    pts.append(pt)

                score_exp(0)
                score_exp(1)
                if prev is not None:
                    enqueue_av(prev)
                for fn in extra:
                    fn()
                for i in range(2, 8):
                    score_exp(i)
                    fq_pop(budget)
                fq_pop(2 * budget)
                return (p, ch, pts)

            # ---- out-projection ------------------------------------------
            CHUNKS = ((0, 512), (512, 256))
            _osb_cache = {}

            def out_group(i, c):
                """Chunk c of row-block i (rows 0..3 path): 3 j-matmuls +
                DVE evict; c==1 flushes the row's 768-wide DMA."""
                c0, cw = CHUNKS[c]
                if c == 0:
                    osb = outsb.tile([128, 768], F32, tag="osb", name=f"osb_{i}")
                    _osb_cache[i] = osb
                else:
                    osb = _osb_cache.pop(i)

                def evict(ps, i=i, c=c, osb=osb):
                    nc.vector.tensor_copy(osb[:, c0:c0 + cw], ps)
                    if c == 1:
                        nc.sync.dma_start(
                            out=out_d[i * 128:(i + 1) * 128, :], in_=osb)
                chain(
                    lambda j, i=i: attT[:, j, i * 128:(i + 1) * 128],
                    lambda j, c0=c0, cw=cw: wo[:, j, c0:c0 + cw],
                    3, cw, evict, f"o_{i}_{c}",
                )

            def out_partial_chain(i, c):
                """j=0,1 of (row-block i, chunk c) into out_partial; c==1
                flushes the row's partial straight to DRAM (the host adds
                the j=2 remainder from out2)."""
                c0, cw = CHUNKS[c]

                def evict(ps, i=i, c=c):
                    nc.vector.tensor_copy(out_partial[:, i - 4, c0:c0 + cw], ps)
                    if c == 1:
                        nc.sync.dma_start(
                            out=out_d[i * 128:(i + 1) * 128, :],
                            in_=out_partial[:, i - 4, :],
                        )
                chain(
                    lambda j, i=i: attT[:, j, i * 128:(i + 1) * 128],
                    lambda j, c0=c0, cw=cw: wo[:, j, c0:c0 + cw],
                    2, cw, evict, f"pp_{i}_{c}",
                )

            def out_final(i):
                """j=2-only remainder of row-block i -> out2 (host adds it
                onto the j=0,1 partials).  Evictions alternate ACT/DVE by
                row so neither engine serializes the tail.  Emits directly
                (tail drain context)."""
                osb = outsb.tile([128, 768], F32, tag="osb", name=f"osbf_{i}")
                r0 = (i - 4) * 128
                for c, (c0, cw) in enumerate(CHUNKS):
                    ps = ps_aux.tile([128, cw], F32, tag="aux", name=f"f_{i}_{c}",
                                     padded_shape=[128, 512])
                    nc.tensor.matmul(
                        ps,
                        attT[:, 2, i * 128:(i + 1) * 128],
                        wo[:, 2, c0:c0 + cw],
                        start=True,
                        stop=True,
                    )
                    if i % 2 == 0:
                        nc.scalar.copy(osb[:, c0:c0 + cw], ps)
                    else:
                        nc.vector.tensor_copy(osb[:, c0:c0 + cw], ps)
                    if i == 7:
                        # flush per chunk so the kernel-end drain only waits
                        # on the short final transfer
                        nc.sync.dma_start(
                            out=out2_d[r0:r0 + 128, c0:c0 + cw],
                            in_=osb[:, c0:c0 + cw],
                        )
                if i != 7:
                    nc.sync.dma_start(out=out2_d[r0:r0 + 128, :], in_=osb)

            # ---- schedule -------------------------------------------------
            # Pre-weave: the minimum projections unit (0,0) needs — q pair 0
            # chunk 0 and k pair 0 both chunks (scores sweep all key blocks).
            # Alternating eviction engines so neither DVE nor ACT gates the
            # first scores.  q chunk 1 flows through the queue.
            qk_group(0, chs=(0,))
            qk_group(1, evict_on_act=True)
            fq_drain()

            # Weave. Query-chunk-0 units first: once (0,0),(1,0),(2,0) are
            # done, output row-blocks 0..3 are fully determined, so their
            # out-projection (and DMA) overlaps the chunk-1 units.  v, the
            # remaining qk projections, AV bursts, and out chains all flow
            # through the fill queue; a full drain before each unit bounds
            # the backlog and guarantees every unit's inputs (its pair's qkT
            # chains, v for the AV bursts) are emitted before its scores.
            # Queue order is deadline order: pair-1 then pair-2 projections
            # (they gate units (1,0)/(2,0)'s scores — the old full drains
            # emitted them at boundaries BEHIND the v chains, opening 3-9us
            # ACT holes), then q pair-0 chunk 1, then v (only consumed by
            # the first AV burst mid-unit-(1,0), which FIFO-orders after it).
            qk_group(2)
            qk_group(3)
            after_qk23 = fq_stat["pushed"]
            qk_group(4)
            qk_group(5)
            after_qk45 = fq_stat["pushed"]
            qk_group(0, chs=(1,))
            for i in range(8):
                v_group(i)
            prev = attention_unit(0, 0, None)
            fq_barrier(after_qk23)
            prev = attention_unit(1, 0, prev)
            fq_barrier(after_qk45)
            prev = attention_unit(2, 0, prev)
            # out row-blocks 0..3 depend only on the ch-0 attT columns; the
            # last of those (pair 2) is transposed inside unit (0,1) by its
            # AV items, so the row chains can fill units (0,1) and (1,1) —
            # without them the late units' fill queues run dry and PE both
            # idles and drops to the cold p-state.
            prev = attention_unit(0, 1, prev, extra=[
                lambda i=i, c=c: out_group(i, c)
                for i in (0, 1) for c in (0, 1)
            ], budget=LATE_BUDGET)
            prev = attention_unit(1, 1, prev, extra=[
                lambda i=i, c=c: out_group(i, c)
                for i in (2, 3) for c in (0, 1)
            ], budget=LATE_BUDGET)
            # rows 4..7 j=0,1 partials read attT pair-0/1 ch-1; pair 1 is
            # only transposed inside unit (2,1) — enqueue them behind its AV
            # items.  The j=2 finals then ride the tail AV's post hooks so
            # each row's add+DMA fires as soon as its attT columns land.
            prev = attention_unit(2, 1, prev, extra=[
                lambda i=i, c=c: out_partial_chain(i, c)
                for i in (4, 5) for c in (0, 1)
            ], budget=LATE_BUDGET)
            out_partial_chain(6, 0)
            out_partial_chain(6, 1)
            enqueue_av_tail(prev, [lambda i=i: out_final(i) for i in (4, 5, 6, 7)],
                            pre=[lambda: out_partial_chain(7, 0),
                                 lambda: out_partial_chain(7, 1)])
            fq_drain()

    nc.compile()
    return nc


_NC_CACHE = {}


def _get_nc():
    if "nc" not in _NC_CACHE:
        _NC_CACHE["nc"] = build_nc()
    return _NC_CACHE["nc"]


def kernel(x, w_qkv, w_out, b_out):
    import ml_dtypes

    def bf16(a):
        return np.ascontiguousarray(
            np.asarray(a, dtype=np.float32).astype(ml_dtypes.bfloat16)
        )

    x = np.asarray(x, dtype=np.float32)
    w_qkv = np.asarray(w_qkv, dtype=np.float32)
    w_out = np.asarray(w_out, dtype=np.float32)
    b_out = np.asarray(b_out, dtype=np.float32)

    nc = _get_nc()
    ones_col = np.ones((128, 64), dtype=np.float32)
    ident = np.eye(128, dtype=np.float32)
    in_maps = []
    for c in range(NCORES):
        b = c // 2
        hs = (c % 2) * HEADS_PER_CORE
        q_cols = w_qkv[:, hs * DH:(hs + 6) * DH]
        k_cols = w_qkv[:, 768 + hs * DH:768 + (hs + 6) * DH]
        # pair-packed: [q_p0 | k_p0 | q_p1 | k_p1 | q_p2 | k_p2], 128 each
        wqk_packed = np.concatenate(
            [blk for p in range(3)
             for blk in (q_cols[:, p * 128:(p + 1) * 128],
                         k_cols[:, p * 128:(p + 1) * 128])],
            axis=1,
        )
        in_maps.append({
            "xT": bf16(x[b].T),
            "w_qk": bf16(wqk_packed),
            "w_v": bf16(w_qkv[:, 1536 + hs * DH:1536 + (hs + 6) * DH]),
            "w_o": bf16(w_out[hs * DH:(hs + 6) * DH, :]),
            "ones_col": bf16(ones_col),
            "ident": bf16(ident),
        })

    res = run_bass_kernel_spmd(nc, in_maps, core_ids=list(range(NCORES))).results

    out = np.empty((4, N, DIM), dtype=np.float32)
    for b in range(4):
        out[b] = res[2 * b]["out"] + res[2 * b + 1]["out"] + b_out
        # rows 512.. left the device as (j01 partials, j2 remainder)
        out[b, N // 2:] += res[2 * b]["out2"] + res[2 * b + 1]["out2"]
    return out
